# revision 1
# baseline (speedup 1.0000x reference)
"""Multi-head attention kernel for Trainium2 (Bass/Tile), 8 NeuronCores.

Problem: nn_MultiHeadAttention
  x [8, 1024, 1024] f32, w_qkv [1024, 3072], b_qkv [3072],
  w_proj [1024, 1024], b_proj [1024]  ->  out [8, 1024, 1024]

  qkv = x @ w_qkv + b_qkv ; split (h, d, 3) interleaved on last dim
  score = q k^T per (b, h);  att = softmax(score, -1) / sqrt(1024)
  out = (att @ v) reshaped @ w_proj + b_proj

Sharding: data-parallel over batch. Each of the 8 cores runs the full
MHA for one batch element; no collectives. Host pre-transposes x and
pre-splits w_qkv so the device program is pure matmul + softmax.

Device-side math per core (all layouts chosen so no on-device transpose
is ever needed):
  qT = (x wq)^T  [(h,d), tok]   lhsT=wq tile, rhs=x^T tile
  kT = (x wk)^T  [(h,d), tok]
  v  = x wv      [tok, (h,d)]   + ones-column per head -> v_aug
  per head: S^T[k,q] = kT.T-slice matmul; E = exp(S^T)
            O'^T[0:64,q] ; O'^T[64,q]=sum_k E  via v_aug ones column
            attoutT = O'[0:64] * (scale / O'[64]) (bcast by PE outer-product)
  out = attoutT.T @ wp + bp   (bias via ones outer-product matmul)
"""

import os

os.environ.setdefault("MYCRO_LOCAL_CACHE", "1")

import numpy as np

import concourse.bass as bass
import concourse.tile as tile
from concourse import bacc, mybir

P = 128
DH = 64  # head dim
F32 = mybir.dt.float32
F32R = mybir.dt.float32r
# matmul-operand dtype: float32r streams at full PE rate (4x fp32);
# values are fp32 bit-patterns rounded by the producing engine
MM = F32R

# full-problem constants
B_FULL = 8
TOK_FULL = 1024
D_FULL = 1024
H_FULL = 16
ATT_SCALE_FULL = 1.0 / 32.0  # 1/sqrt(1024), applied after softmax
N_CORES = 8


def _chunks(total, step=512):
    return [(s, min(step, total - s)) for s in range(0, total, step)]


def build(nc, TOK, D, H, att_scale):
    """Emit the one-core MHA program (one batch element).

    DRAM inputs (host pre-laid-out):
      x        [P, KT*TOK]   [p, kt, t] = x[t, kt*P + p]   (x^T, kt-tiled)
      wq/wk/wv/wp [P, KT*D]  [p, kt, n] = w[kt*P + p, n]
      bq/bk    [P, NPAIR]    [p, m] = b[m*P + p]
      bv/bp    [1, D]
    Output: out [TOK, D]
    """
    assert D == H * DH and D % P == 0 and TOK % P == 0 and H % 2 == 0
    KT = D // P       # contraction tiles over the model dim
    MT = TOK // P     # token tiles
    NPAIR = H // 2    # head pairs (== D // P)
    VW = H * (DH + 1)  # v_aug row width: per head [v | 1]
    EXP = mybir.ActivationFunctionType.Exp

    x_d = nc.dram_tensor("x", [P, KT * TOK], MM, kind="ExternalInput")
    w_d = {}
    for nm in ("wq", "wk", "wv", "wp"):
        w_d[nm] = nc.dram_tensor(nm, [P, KT * D], MM, kind="ExternalInput")
    bq_d = nc.dram_tensor("bq", [P, NPAIR], F32, kind="ExternalInput")
    bk_d = nc.dram_tensor("bk", [P, NPAIR], F32, kind="ExternalInput")
    bv_d = nc.dram_tensor("bv", [1, D], MM, kind="ExternalInput")
    bp_d = nc.dram_tensor("bp", [1, D], MM, kind="ExternalInput")
    out_d = nc.dram_tensor("out", [TOK, D], F32, kind="ExternalOutput")

    with tile.TileContext(nc) as tc:
        with (
            tc.tile_pool(name="sing", bufs=1) as sing,
            tc.tile_pool(name="psA", bufs=2, space="PSUM") as psA,
            tc.tile_pool(name="psB", bufs=2, space="PSUM") as psB,
            tc.tile_pool(name="ebuf", bufs=4) as ebuf,
            tc.tile_pool(name="rbuf", bufs=2) as rbuf,
            tc.tile_pool(name="outp", bufs=2) as outp,
        ):
            # ---------------- persistent SBUF ----------------
            # memset cannot target f32r; stage in f32, round via DVE copy
            cst_sb = sing.tile([1, P + DH], F32, tag="cst")
            nc.vector.memset(cst_sb[:, 0:P], 1.0)
            nc.vector.memset(cst_sb[:, P : P + DH], att_scale)
            ones_sb = sing.tile([1, P], MM, tag="ones")
            nc.vector.tensor_copy(out=ones_sb, in_=cst_sb[:, 0:P])
            scl_sb = sing.tile([1, DH], F32, tag="scl")
            nc.vector.tensor_copy(out=scl_sb, in_=cst_sb[:, P : P + DH])
            vones_sb = sing.tile([P, MT * H], F32, tag="vones")
            nc.vector.memset(vones_sb, 1.0)

            bq_sb = sing.tile([P, NPAIR], F32, tag="bq")
            nc.sync.dma_start(out=bq_sb, in_=bq_d[:, :])
            bk_sb = sing.tile([P, NPAIR], F32, tag="bk")
            nc.sync.dma_start(out=bk_sb, in_=bk_d[:, :])
            bv_sb = sing.tile([1, D], MM, tag="bv")
            nc.sync.dma_start(out=bv_sb, in_=bv_d[:, :])
            bp_sb = sing.tile([1, D], MM, tag="bp")
            nc.sync.dma_start(out=bp_sb, in_=bp_d[:, :])

            v_sb = sing.tile([P, MT, VW], MM, tag="v")     # v_aug
            # ones columns (denominator accumulators), rounded f32->f32r
            nc.vector.tensor_copy(
                out=v_sb[:, :, :]
                .rearrange("p m (h e) -> p m h e", e=DH + 1)[:, :, :, DH],
                in_=vones_sb[:, :].rearrange("p (m h) -> p m h", h=H),
            )
            qT_sb = sing.tile([P, NPAIR, TOK], MM, tag="qT")
            kT_sb = sing.tile([P, NPAIR, TOK], MM, tag="kT")
            ao_sb = sing.tile([P, NPAIR, TOK], MM, tag="ao")  # attout^T

            with tc.tile_pool(name="xp", bufs=1) as xp:
                x_sb = xp.tile([P, KT * TOK], MM, tag="x")
                nc.sync.dma_start(out=x_sb, in_=x_d[:, :])
                x3 = x_sb[:, :].rearrange("p (kt t) -> p kt t", t=TOK)

                # ---------------- V phase: v = x wv + bv (natural) ----
                # stream wv in D-column halves to bound SBUF
                with tc.tile_pool(name="wvp", bufs=1) as wvp:
                    for c0, cw in _chunks(D, 256):
                        wv_sb = wvp.tile([P, KT, 256], MM, tag="wv")
                        nc.sync.dma_start(
                            out=wv_sb[:, :, 0:cw],
                            in_=w_d["wv"][:, :]
                            .rearrange("p (kt n) -> p kt n", n=D)[:, :, c0 : c0 + cw],
                        )
                        for mt in range(MT):
                            ps_v = psA.tile([P, 512], F32, tag="psA")
                            for kt in range(KT):
                                nc.tensor.matmul(
                                    ps_v[:, 0:cw],
                                    lhsT=x3[:, kt, mt * P : (mt + 1) * P],
                                    rhs=wv_sb[:, kt, 0:cw],
                                    start=(kt == 0),
                                    stop=False,
                                )
                            # + bv by ones outer-product
                            nc.tensor.matmul(
                                ps_v[:, 0:cw],
                                lhsT=ones_sb[0:1, 0:P],
                                rhs=bv_sb[0:1, c0 : c0 + cw],
                                start=False,
                                stop=True,
                            )
                            # scatter heads into v_aug (65-stride)
                            nh = cw // DH
                            h0 = c0 // DH
                            nc.vector.tensor_copy(
                                out=v_sb[:, mt, :]
                                .rearrange("p (h e) -> p h e", e=DH + 1)[
                                    :, h0 : h0 + nh, 0:DH
                                ],
                                in_=ps_v[:, 0:cw].rearrange(
                                    "p (h d) -> p h d", d=DH
                                ),
                            )

                # ---------------- Q phase: qT = (x wq)^T + bq --------
                # ---------------- K phase: kT = (x wk)^T + bk --------
                for wname, dst_sb, b_sb in (
                    ("wq", qT_sb, bq_sb),
                    ("wk", kT_sb, bk_sb),
                ):
                    with tc.tile_pool(name=wname + "p", bufs=1) as wqp:
                        half = min(2, NPAIR)
                        for pg0 in range(0, NPAIR, half):
                            pg1 = min(pg0 + half, NPAIR)
                            wq_sb = wqp.tile([P, KT, half * P], MM, tag="w")
                            nc.sync.dma_start(
                                out=wq_sb[:, :, 0 : (pg1 - pg0) * P],
                                in_=w_d[wname][:, :]
                                .rearrange("p (kt n) -> p kt n", n=D)[
                                    :, :, pg0 * P : pg1 * P
                                ],
                            )
                            for pp in range(pg0, pg1):
                                ps_q = psA.tile([P, TOK], F32, tag="psA")
                                for c0, cw in _chunks(TOK, 512):
                                    for kt in range(KT):
                                        nc.tensor.matmul(
                                            ps_q[:, c0 : c0 + cw],
                                            lhsT=wq_sb[
                                                :,
                                                kt,
                                                (pp - pg0) * P : (pp - pg0 + 1) * P,
                                            ],
                                            rhs=x3[:, kt, c0 : c0 + cw],
                                            start=(kt == 0),
                                            stop=(kt == KT - 1),
                                        )
                                    nc.vector.tensor_scalar_add(
                                        out=dst_sb[:, pp, c0 : c0 + cw],
                                        in0=ps_q[:, c0 : c0 + cw],
                                        scalar1=b_sb[:, pp : pp + 1],
                                    )

            # ---------------- attention, per head pair ----------------
            # scores^T[k,q] per head; E=exp; O'^T accum over k tiles;
            # row DH of O'^T is the softmax denominator (ones column).
            def emit_scores(p, kb, ps_e, ps_o):
                for base, ps in ((0, ps_e), (DH, ps_o)):
                    for c0, cw in _chunks(TOK, 512):
                        nc.tensor.matmul(
                            ps[:, c0 : c0 + cw],
                            lhsT=kT_sb[
                                base : base + DH, p, kb * P : (kb + 1) * P
                            ],
                            rhs=qT_sb[base : base + DH, p, c0 : c0 + cw],
                            start=True,
                            stop=True,
                        )

            for p in range(NPAIR):
                ps_oe = psB.tile([DH + 1, TOK], F32, tag="psB")
                ps_oo = psB.tile([DH + 1, TOK], F32, tag="psB")
                sc_tiles = {}
                e_tiles = {}

                def emit_sc_exp(kb):
                    ps_e = psA.tile([P, TOK], F32, tag="psA")
                    ps_o = psA.tile([P, TOK], F32, tag="psA")
                    emit_scores(p, kb, ps_e, ps_o)
                    ee = ebuf.tile([P, TOK], MM, tag="E")
                    eo = ebuf.tile([P, TOK], MM, tag="E")
                    nc.scalar.activation(out=ee, in_=ps_e, func=EXP)
                    nc.scalar.activation(out=eo, in_=ps_o, func=EXP)
                    sc_tiles[kb] = (ps_e, ps_o)
                    e_tiles[kb] = (ee, eo)

                emit_sc_exp(0)
                for kb in range(MT):
                    if kb + 1 < MT:
                        emit_sc_exp(kb + 1)
                    ee, eo = e_tiles.pop(kb)
                    del sc_tiles[kb]
                    for hoff, ps_out, et in ((0, ps_oe, ee), (1, ps_oo, eo)):
                        hh = 2 * p + hoff
                        for c0, cw in _chunks(TOK, 512):
                            nc.tensor.matmul(
                                ps_out[:, c0 : c0 + cw],
                                lhsT=v_sb[
                                    :, kb, hh * (DH + 1) : (hh + 1) * (DH + 1)
                                ],
                                rhs=et[:, c0 : c0 + cw],
                                start=(kb == 0),
                                stop=(kb == MT - 1),
                                skip_group_check=True,
                            )

                # normalize: attout^T = O'[0:DH] * (scale / O'[DH])
                r_sb = rbuf.tile([1, 2, TOK], F32, tag="R")
                nc.vector.reciprocal(r_sb[0:1, 0, :], ps_oe[DH : DH + 1, :])
                nc.vector.reciprocal(r_sb[0:1, 1, :], ps_oo[DH : DH + 1, :])
                for ri, ps_o in ((0, ps_oe), (1, ps_oo)):
                    bc = psA.tile([P, TOK], F32, tag="psA")
                    for c0, cw in _chunks(TOK, 512):
                        nc.tensor.matmul(
                            bc[0:DH, c0 : c0 + cw],
                            lhsT=scl_sb[0:1, 0:DH],
                            rhs=r_sb[0:1, ri, c0 : c0 + cw],
                            start=True,
                            stop=True,
                        )
                    # DVE may read only one PSUM operand per instruction
                    bc_sb = ebuf.tile([P, TOK], MM, tag="E")
                    nc.vector.tensor_copy(
                        out=bc_sb[0:DH, :], in_=bc[0:DH, :]
                    )
                    nc.vector.tensor_mul(
                        out=ao_sb[
                            ri * DH : (ri + 1) * DH, p, :
                        ],
                        in0=ps_o[0:DH, :],
                        in1=bc_sb[0:DH, :],
                    )

            # ---------------- projection: out = attout wp + bp -------
            # stream wp in 512-column halves; per half, all token tiles
            with tc.tile_pool(name="wpp", bufs=2) as wpp:
                for c0, cw in _chunks(D, 512):
                    wp_sb = wpp.tile([P, KT, 512], MM, tag="wp")
                    nc.sync.dma_start(
                        out=wp_sb[:, :, 0:cw],
                        in_=w_d["wp"][:, :]
                        .rearrange("p (kt n) -> p kt n", n=D)[:, :, c0 : c0 + cw],
                    )
                    for mt in range(MT):
                        ps_p = psB.tile([P, 512], F32, tag="psB")
                        for kt in range(KT):
                            nc.tensor.matmul(
                                ps_p[:, 0:cw],
                                lhsT=ao_sb[:, kt, mt * P : (mt + 1) * P],
                                rhs=wp_sb[:, kt, 0:cw],
                                start=(kt == 0),
                                stop=False,
                            )
                        nc.tensor.matmul(
                            ps_p[:, 0:cw],
                            lhsT=ones_sb[0:1, 0:P],
                            rhs=bp_sb[0:1, c0 : c0 + cw],
                            start=False,
                            stop=True,
                        )
                        o_sb = outp.tile([P, 512], F32, tag="o")
                        nc.vector.tensor_copy(
                            out=o_sb[:, 0:cw], in_=ps_p[:, 0:cw]
                        )
                        nc.sync.dma_start(
                            out=out_d[mt * P : (mt + 1) * P, c0 : c0 + cw],
                            in_=o_sb[:, 0:cw],
                        )

    return nc


# ---------------------------------------------------------------------------
# host-side layout prep
# ---------------------------------------------------------------------------

def _round_f32r(x):
    """RNE to f32r's 11-explicit-mantissa-bit grid (matches HW rounding)."""
    u = np.ascontiguousarray(x, np.float32).view(np.uint32)
    u = ((u + np.uint32(1 << 11)) >> 12) << 12
    return u.view(np.float32)


def host_prep_shared(w_qkv, b_qkv, w_proj, b_proj, D, H):
    """Split/retile the weights once for all cores."""
    KT = D // P
    NPAIR = H // 2

    def tile_w(w):  # [D, N] -> [P, KT*N]
        N = w.shape[1]
        return _round_f32r(
            w.reshape(KT, P, N).transpose(1, 0, 2).reshape(P, KT * N)
        )

    wq3 = w_qkv.reshape(D, H, DH, 3)
    out = {
        "wq": tile_w(np.ascontiguousarray(wq3[:, :, :, 0].reshape(D, D))),
        "wk": tile_w(np.ascontiguousarray(wq3[:, :, :, 1].reshape(D, D))),
        "wv": tile_w(np.ascontiguousarray(wq3[:, :, :, 2].reshape(D, D))),
        "wp": tile_w(np.ascontiguousarray(w_proj)),
    }
    b3 = b_qkv.reshape(H, DH, 3)
    bq = np.ascontiguousarray(b3[:, :, 0].reshape(D))
    bk = np.ascontiguousarray(b3[:, :, 1].reshape(D))
    bv = np.ascontiguousarray(b3[:, :, 2].reshape(D))
    out["bq"] = np.ascontiguousarray(bq.reshape(NPAIR, P).T).astype(np.float32)
    out["bk"] = np.ascontiguousarray(bk.reshape(NPAIR, P).T).astype(np.float32)
    out["bv"] = _round_f32r(bv.reshape(1, D))
    out["bp"] = _round_f32r(np.asarray(b_proj, np.float32).reshape(1, D))
    return out


def host_prep_x(x_b, TOK, D):
    """One batch element [TOK, D] -> x^T tiled [P, KT*TOK]."""
    KT = D // P
    xT = np.ascontiguousarray(np.asarray(x_b, np.float32).T)  # [D, TOK]
    return _round_f32r(
        xT.reshape(KT, P, TOK).transpose(1, 0, 2).reshape(P, KT * TOK)
    )


# ---------------------------------------------------------------------------
# entry point
# ---------------------------------------------------------------------------

_BUILT = {}


def _get_nc(TOK, D, H, att_scale):
    key = (TOK, D, H, att_scale)
    if key not in _BUILT:
        nc = bacc.Bacc(
            "TRN2",
            target_bir_lowering=False,
            debug=False,
            dynamic_dma_scratch_size=512,
        )
        build(nc, TOK, D, H, att_scale)
        nc.compile()
        nc.finalize()
        _BUILT[key] = nc
    return _BUILT[key]


def kernel(x, w_qkv, b_qkv, w_proj, b_proj):
    from concourse.bass_utils import run_bass_kernel_spmd

    x = np.asarray(x, np.float32)
    B, TOK, D = x.shape
    H = H_FULL
    shared = host_prep_shared(
        np.asarray(w_qkv, np.float32),
        np.asarray(b_qkv, np.float32),
        np.asarray(w_proj, np.float32),
        np.asarray(b_proj, np.float32),
        D,
        H,
    )
    in_maps = []
    for b in range(B):
        m = dict(shared)
        m["x"] = host_prep_x(x[b], TOK, D)
        in_maps.append(m)

    nc = _get_nc(TOK, D, H, ATT_SCALE_FULL)
    res = run_bass_kernel_spmd(nc, in_maps, list(range(N_CORES)))
    out = np.stack([res.results[b]["out"] for b in range(B)], axis=0)
    return out.astype(np.float32)



# revision 6
# speedup vs baseline: 1.5664x; 1.5664x over previous
"""Multi-head attention kernel for Trainium2 (Bass/Tile), 8 NeuronCores.

Problem: nn_MultiHeadAttention
  x [8, 1024, 1024] f32, w_qkv [1024, 3072], b_qkv [3072],
  w_proj [1024, 1024], b_proj [1024]  ->  out [8, 1024, 1024]

  qkv = x @ w_qkv + b_qkv ; split (h, d, 3) interleaved on last dim
  score = q k^T per (b, h);  att = softmax(score, -1) / sqrt(1024)
  out = (att @ v) reshaped @ w_proj + b_proj

Sharding: data-parallel over batch. Each of the 8 cores runs the full
MHA for one batch element; no collectives.

v2 design (post-trace): the v1 kernel ran at 680us with the PE
clock-gated to 1.2GHz for 2/3 of the span (HAM re-throttle during
serial per-pair stalls) and 104us of single-partition DVE RECIPROCAL.
This version:
  - keeps the PE warm: attention is an ACT-paced conveyor (scores ->
    exp -> att@V per k-tile, ping-pong PSUM), with the NEXT pair's
    Q/K projections interleaved into the same span so the PE never
    idles long enough to re-throttle.
  - scores run as two concurrent row-tiled (64x128) matmuls: even
    head on PE tile T0 (SBUF rows 0:64), odd head on T8 (rows 64:128).
  - softmax denominators come free from a 65th "32.0" column in the
    V operand (folds the 1/sqrt(D) post-scale); their reciprocals are
    computed 2 rows at a time with reciprocal_approx_fast (~5x faster,
    128 partitions wide) and broadcast across partitions with one
    K=2 indicator matmul.
  - all weight DMA slices are contiguous per partition (per-pair /
    per-chunk major DRAM layout).

Device-side math per core (no on-device transpose anywhere):
  qT = (x wq)^T  [(h,d), tok]   kT likewise
  v_aug = [x wv + bv | 32.0] per head   [tok, h*(d+1)]
  per head pair, per k-tile: S^T[k,q] = kT.T-slice @ qT-slice (T0/T8)
     E = exp(S^T); O'^T[0:64,q] += v_aug.T @ E; O'^T[64,q] = 32*denom
  ao^T = O' * (1/(32*denom))  (recip via DVE approx, bcast via PE)
  out = ao^T.T @ wp + bp
"""

import os

os.environ.setdefault("MYCRO_LOCAL_CACHE", "1")

import numpy as np

import concourse.bass as bass
import concourse.tile as tile
from concourse import bacc, mybir

P = 128
DH = 64  # head dim
F32 = mybir.dt.float32
F32R = mybir.dt.float32r
# matmul-operand dtype: float32r streams at full PE rate when the
# moving free dim is >= 256
MM = F32R

# full-problem constants
B_FULL = 8
TOK_FULL = 1024
D_FULL = 1024
H_FULL = 16
ATT_SCALE_FULL = 1.0 / 32.0  # 1/sqrt(1024), applied after softmax
N_CORES = 8


def build(nc, TOK, D, H, att_scale):
    """Emit the one-core MHA program (one batch element).

    DRAM inputs (host pre-laid-out, all slices contiguous/partition):
      x        [P, KT*TOK]        [p][kt][t] = x[t, kt*P + p]
      wq, wk   [P, NPAIR*KT*P]    [p][pair][kt][n]; cols n = pair block
      wv, wp   [P, NVCH*KT*VCH]   [p][c][kt][n];  cols n = chunk block
      bq, bk   [P, NPAIR]         [p][pair] = b[pair*P + p]
      bv, bp   [1, D]
    Output: out [TOK, D] f32
    """
    assert D == H * DH and D % P == 0 and TOK % P == 0 and H % 2 == 0
    KT = D // P        # contraction tiles over the model dim
    MT = TOK // P      # token tiles (also the k-tiles of attention)
    NPAIR = H // 2     # head pairs
    VW = H * (DH + 1)  # v_aug row width: per head [v | aug]
    QCH = min(512, TOK)   # moving-chunk width for scores / att@V / QK
    NQH = TOK // QCH
    VCH = min(512, D)     # column chunk for V / proj weight streaming
    NVCH = D // VCH
    AUG = 1.0 / att_scale  # 32.0: folded post-softmax scale
    EXP = mybir.ActivationFunctionType.Exp
    assert MT % 2 == 0

    x_d = nc.dram_tensor("x", [P, KT * TOK], MM, kind="ExternalInput")
    wq_d = nc.dram_tensor("wq", [P, NPAIR * KT * P], MM, kind="ExternalInput")
    wk_d = nc.dram_tensor("wk", [P, NPAIR * KT * P], MM, kind="ExternalInput")
    wv_d = nc.dram_tensor("wv", [P, NVCH * KT * VCH], MM, kind="ExternalInput")
    wp_d = nc.dram_tensor("wp", [P, NVCH * KT * VCH], MM, kind="ExternalInput")
    bq_d = nc.dram_tensor("bq", [P, NPAIR], F32, kind="ExternalInput")
    bk_d = nc.dram_tensor("bk", [P, NPAIR], F32, kind="ExternalInput")
    bv_d = nc.dram_tensor("bv", [1, D], MM, kind="ExternalInput")
    bp_d = nc.dram_tensor("bp", [1, D], MM, kind="ExternalInput")
    out_d = nc.dram_tensor("out", [TOK, D], F32, kind="ExternalOutput")

    with tile.TileContext(nc) as tc:
        with (
            tc.tile_pool(name="sing", bufs=1) as sing,
            tc.tile_pool(name="psS", bufs=2, space="PSUM") as psS,
            tc.tile_pool(name="psO", bufs=4, space="PSUM") as psO,
            tc.tile_pool(name="ebuf", bufs=4) as ebuf,
            tc.tile_pool(name="qkp", bufs=2) as qkp,
            tc.tile_pool(name="wqkp", bufs=2) as wqkp,
            tc.tile_pool(name="rbuf", bufs=2) as rbuf,
            tc.tile_pool(name="outp", bufs=2) as outp,
        ):
            # ---------------- persistent SBUF ----------------
            # memset cannot target f32r; stage in f32, round via DVE copy
            cst_sb = sing.tile([2, P], F32, tag="cst")
            nc.vector.memset(cst_sb, 1.0)
            ones_sb = sing.tile([1, P], MM, tag="ones")
            nc.vector.tensor_copy(out=ones_sb, in_=cst_sb[0:1, :])
            vones_sb = sing.tile([P, MT * H], F32, tag="vones")
            nc.vector.memset(vones_sb, AUG)

            bq_sb = sing.tile([P, NPAIR], F32, tag="bq")
            nc.sync.dma_start(out=bq_sb, in_=bq_d[:, :])
            bk_sb = sing.tile([P, NPAIR], F32, tag="bk")
            nc.sync.dma_start(out=bk_sb, in_=bk_d[:, :])
            bv_sb = sing.tile([1, D], MM, tag="bv")
            nc.sync.dma_start(out=bv_sb, in_=bv_d[:, :])
            bp_sb = sing.tile([1, D], MM, tag="bp")
            nc.sync.dma_start(out=bp_sb, in_=bp_d[:, :])

            v_sb = sing.tile([P, MT, VW], MM, tag="v")   # v_aug
            # aug columns (denominator accumulators) = 1/att_scale
            nc.vector.tensor_copy(
                out=v_sb[:, :, :]
                .rearrange("p m (h e) -> p m h e", e=DH + 1)[:, :, :, DH],
                in_=vones_sb[:, :].rearrange("p (m h) -> p m h", h=H),
            )
            ao_sb = sing.tile([P, NPAIR, TOK], MM, tag="ao")  # attout^T

            x_sb = sing.tile([P, KT, TOK], MM, tag="x")
            half = KT // 2
            nc.sync.dma_start(
                out=x_sb[:, 0:half, :], in_=x_d[:, 0 : half * TOK]
            )
            nc.sync.dma_start(
                out=x_sb[:, half:KT, :], in_=x_d[:, half * TOK : KT * TOK]
            )
            x3 = x_sb

            wv3 = wv_d[:, :].rearrange("p (c kt n) -> p c kt n", c=NVCH, kt=KT)
            wp3 = wp_d[:, :].rearrange("p (c kt n) -> p c kt n", c=NVCH, kt=KT)
            wq3 = wq_d[:, :].rearrange("p (pr kt n) -> p pr kt n", pr=NPAIR, kt=KT)
            wk3 = wk_d[:, :].rearrange("p (pr kt n) -> p pr kt n", pr=NPAIR, kt=KT)

            # ---------------- Q/K projection task ----------------
            # qT/kT for one pair: [P rows = (even|odd head dims), TOK]
            qT = {}
            kT = {}

            def load_wqk(p):
                wq_sb = wqkp.tile([P, KT, P], MM, tag="wq")
                nc.sync.dma_start(
                    out=wq_sb, in_=wq3[:, p, :, :]
                )
                wk_sb = wqkp.tile([P, KT, P], MM, tag="wk")
                nc.sync.dma_start(
                    out=wk_sb, in_=wk3[:, p, :, :]
                )
                return wq_sb, wk_sb

            def emit_qk_task(p, which, w_sb, c0, cw):
                """One accumulation task: (x @ w_pair)^T chunk + bias."""
                if which == "q":
                    if p not in qT:
                        qT[p] = qkp.tile([P, TOK], MM, tag="qT", name=f"qT{p}")
                    dst, b_sb = qT[p], bq_sb
                else:
                    if p not in kT:
                        kT[p] = qkp.tile([P, TOK], MM, tag="kT", name=f"kT{p}")
                    dst, b_sb = kT[p], bk_sb
                ps = psO.tile([P, QCH], F32, tag="O")
                for kt in range(KT):
                    nc.tensor.matmul(
                        ps[:, 0:cw],
                        lhsT=w_sb[:, kt, :],
                        rhs=x3[:, kt, c0 : c0 + cw],
                        start=(kt == 0),
                        stop=(kt == KT - 1),
                    )
                nc.vector.tensor_scalar_add(
                    out=dst[:, c0 : c0 + cw],
                    in0=ps[:, 0:cw],
                    scalar1=b_sb[:, p : p + 1],
                )

            # ---------------- V phase: v_aug = [x wv + bv | AUG] --------
            wqk0 = load_wqk(0)
            with tc.tile_pool(name="wvp", bufs=2) as wvp:
                for c in range(NVCH):
                    wv_sb = wvp.tile([P, KT, VCH], MM, tag="wv")
                    nc.sync.dma_start(out=wv_sb, in_=wv3[:, c, :, :])
                    for mt in range(MT):
                        ps_v = psO.tile([P, QCH], F32, tag="O")
                        for kt in range(KT):
                            nc.tensor.matmul(
                                ps_v[:, 0:VCH],
                                lhsT=x3[:, kt, mt * P : (mt + 1) * P],
                                rhs=wv_sb[:, kt, :],
                                start=(kt == 0),
                                stop=False,
                            )
                        nc.tensor.matmul(
                            ps_v[:, 0:VCH],
                            lhsT=ones_sb[0:1, 0:P],
                            rhs=bv_sb[0:1, c * VCH : (c + 1) * VCH],
                            start=False,
                            stop=True,
                        )
                        # scatter heads into v_aug (DH+1 stride)
                        nh = VCH // DH
                        h0 = c * VCH // DH
                        nc.vector.tensor_copy(
                            out=v_sb[:, mt, :]
                            .rearrange("p (h e) -> p h e", e=DH + 1)[
                                :, h0 : h0 + nh, 0:DH
                            ],
                            in_=ps_v[:, 0:VCH].rearrange(
                                "p (h d) -> p h d", d=DH
                            ),
                        )
                    # interleave pair-0 Q/K projections into the V phase
                    if c == 0:
                        for c0 in range(0, TOK, QCH):
                            emit_qk_task(0, "q", wqk0[0], c0, QCH)
                    if c == NVCH - 1 or NVCH == 1:
                        for c0 in range(0, TOK, QCH):
                            emit_qk_task(0, "k", wqk0[1], c0, QCH)

            # ---------------- attention, per head pair ----------------
            for p in range(NPAIR):
                wqk_next = load_wqk(p + 1) if p + 1 < NPAIR else None
                for qh in range(NQH):
                    q0 = qh * QCH
                    ps_oe = psO.tile([P, QCH], F32, tag="O")
                    ps_oo = psO.tile([P, QCH], F32, tag="O")
                    for g in range(0, MT, 2):
                        kbs = (g, g + 1)
                        s_t = {}
                        e_t = {}
                        for kb in kbs:
                            st = psS.tile([P, 2 * QCH], F32, tag="S")
                            # even head: PE tile T0 (SBUF rows 0:64)
                            # odd head: T8 (rows 64:128) -- concurrent
                            nc.tensor.matmul(
                                st[:, 0:QCH],
                                lhsT=kT[p][0:DH, kb * P : (kb + 1) * P],
                                rhs=qT[p][0:DH, q0 : q0 + QCH],
                                start=True,
                                stop=True,
                            )
                            nc.tensor.matmul(
                                st[:, QCH : 2 * QCH],
                                lhsT=kT[p][DH:P, kb * P : (kb + 1) * P],
                                rhs=qT[p][DH:P, q0 : q0 + QCH],
                                start=True,
                                stop=True,
                            )
                            s_t[kb] = st
                        for kb in kbs:
                            et = ebuf.tile([P, 2 * QCH], MM, tag="E")
                            nc.scalar.activation(
                                out=et, in_=s_t[kb], func=EXP
                            )
                            e_t[kb] = et
                        for kb in kbs:
                            et = e_t[kb]
                            for hoff, ps_out in ((0, ps_oe), (1, ps_oo)):
                                hh = 2 * p + hoff
                                nc.tensor.matmul(
                                    ps_out[0 : DH + 1, :],
                                    lhsT=v_sb[
                                        :, kb,
                                        hh * (DH + 1) : (hh + 1) * (DH + 1),
                                    ],
                                    rhs=et[:, hoff * QCH : (hoff + 1) * QCH],
                                    start=(kb == 0),
                                    stop=(kb == MT - 1),
                                    skip_group_check=True,
                                )
                        # interleave next pair's Q (during qh 0) and K
                        # (during qh NQH-1) projection tasks
                        if wqk_next is not None and g + 2 >= MT:
                            if qh == 0:
                                for c0 in range(0, TOK, QCH):
                                    emit_qk_task(
                                        p + 1, "q", wqk_next[0], c0, QCH
                                    )
                            if qh == NQH - 1:
                                for c0 in range(0, TOK, QCH):
                                    emit_qk_task(
                                        p + 1, "k", wqk_next[1], c0, QCH
                                    )

                    # ---- normalize: ao^T = O'[0:DH] / O'[DH] ----
                    rq_e = rbuf.tile([1, QCH], F32, tag="rqe")
                    nc.vector.tensor_copy(
                        out=rq_e, in_=ps_oe[DH : DH + 1, :]
                    )
                    rq_o = rbuf.tile([1, QCH], F32, tag="rqo")
                    nc.vector.tensor_copy(
                        out=rq_o, in_=ps_oo[DH : DH + 1, :]
                    )
                    rr_e = rbuf.tile([1, QCH], F32, tag="rre")
                    nc.vector.reciprocal_approx_fast(out=rr_e, in_=rq_e)
                    rr_o = rbuf.tile([1, QCH], F32, tag="rro")
                    nc.vector.reciprocal_approx_fast(out=rr_o, in_=rq_o)
                    rc_e = rbuf.tile([1, QCH], MM, tag="rce")
                    nc.vector.tensor_copy(out=rc_e, in_=rr_e)
                    rc_o = rbuf.tile([1, QCH], MM, tag="rco")
                    nc.vector.tensor_copy(out=rc_o, in_=rr_o)
                    # partition-broadcast via two K=1 ones-column matmuls
                    # (dst base partition must be 0; use column ranges)
                    bc = psS.tile([P, 2 * QCH], F32, tag="S")
                    nc.tensor.matmul(
                        bc[0:DH, 0:QCH],
                        lhsT=ones_sb[0:1, 0:DH],
                        rhs=rc_e[0:1, :],
                        start=True,
                        stop=True,
                    )
                    nc.tensor.matmul(
                        bc[0:DH, QCH : 2 * QCH],
                        lhsT=ones_sb[0:1, 0:DH],
                        rhs=rc_o[0:1, :],
                        start=True,
                        stop=True,
                    )
                    bcs = rbuf.tile([P, 2 * QCH], MM, tag="bcs")
                    nc.vector.tensor_copy(
                        out=bcs[0:DH, :], in_=bc[0:DH, :]
                    )
                    nc.vector.tensor_mul(
                        out=ao_sb[0:DH, p, q0 : q0 + QCH],
                        in0=ps_oe[0:DH, :],
                        in1=bcs[0:DH, 0:QCH],
                    )
                    nc.vector.tensor_mul(
                        out=ao_sb[DH:P, p, q0 : q0 + QCH],
                        in0=ps_oo[0:DH, :],
                        in1=bcs[0:DH, QCH : 2 * QCH],
                    )

            # ---------------- projection: out = ao^T.T wp + bp -------
            with tc.tile_pool(name="wpp", bufs=2) as wpp:
                for c in range(NVCH):
                    wp_sb = wpp.tile([P, KT, VCH], MM, tag="wp")
                    nc.sync.dma_start(out=wp_sb, in_=wp3[:, c, :, :])
                    for mt in range(MT):
                        ps_p = psO.tile([P, QCH], F32, tag="O")
                        for kt in range(KT):
                            nc.tensor.matmul(
                                ps_p[:, 0:VCH],
                                lhsT=ao_sb[:, kt, mt * P : (mt + 1) * P],
                                rhs=wp_sb[:, kt, :],
                                start=(kt == 0),
                                stop=False,
                            )
                        nc.tensor.matmul(
                            ps_p[:, 0:VCH],
                            lhsT=ones_sb[0:1, 0:P],
                            rhs=bp_sb[0:1, c * VCH : (c + 1) * VCH],
                            start=False,
                            stop=True,
                        )
                        o_sb = outp.tile([P, VCH], F32, tag="o")
                        nc.vector.tensor_copy(
                            out=o_sb[:, 0:VCH], in_=ps_p[:, 0:VCH]
                        )
                        nc.sync.dma_start(
                            out=out_d[
                                mt * P : (mt + 1) * P,
                                c * VCH : (c + 1) * VCH,
                            ],
                            in_=o_sb[:, 0:VCH],
                        )

    return nc


# ---------------------------------------------------------------------------
# host-side layout prep
# ---------------------------------------------------------------------------

def _round_f32r(x):
    """RNE to f32r's 11-explicit-mantissa-bit grid (matches HW rounding)."""
    u = np.ascontiguousarray(x, np.float32).view(np.uint32)
    u = ((u + np.uint32(1 << 11)) >> 12) << 12
    return u.view(np.float32)


def _tile_rows(w):
    """[D, N] -> [P, (D//P) * N] with [p][kt][n] layout."""
    Dd, N = w.shape
    KT = Dd // P
    return np.ascontiguousarray(
        w.reshape(KT, P, N).transpose(1, 0, 2).reshape(P, KT * N)
    )


def host_prep_shared(w_qkv, b_qkv, w_proj, b_proj, D, H):
    """Split/retile the weights once for all cores."""
    NPAIR = H // 2
    VCH = min(512, D)
    NVCH = D // VCH

    wq3 = w_qkv.reshape(D, H, DH, 3)
    wq = np.ascontiguousarray(wq3[:, :, :, 0].reshape(D, D))
    wk = np.ascontiguousarray(wq3[:, :, :, 1].reshape(D, D))
    wv = np.ascontiguousarray(wq3[:, :, :, 2].reshape(D, D))
    wp = np.ascontiguousarray(np.asarray(w_proj, np.float32))

    def pair_major(w):  # [D, D] -> [P, NPAIR*KT*P], pair-block major
        blocks = [
            _tile_rows(w[:, p * P : (p + 1) * P]) for p in range(NPAIR)
        ]
        return _round_f32r(np.concatenate(blocks, axis=1))

    def chunk_major(w):  # [D, D] -> [P, NVCH*KT*VCH], chunk major
        blocks = [
            _tile_rows(w[:, c * VCH : (c + 1) * VCH]) for c in range(NVCH)
        ]
        return _round_f32r(np.concatenate(blocks, axis=1))

    out = {
        "wq": pair_major(wq),
        "wk": pair_major(wk),
        "wv": chunk_major(wv),
        "wp": chunk_major(wp),
    }
    b3 = np.asarray(b_qkv, np.float32).reshape(H, DH, 3)
    bq = np.ascontiguousarray(b3[:, :, 0].reshape(D))
    bk = np.ascontiguousarray(b3[:, :, 1].reshape(D))
    bv = np.ascontiguousarray(b3[:, :, 2].reshape(D))
    out["bq"] = np.ascontiguousarray(bq.reshape(NPAIR, P).T).astype(np.float32)
    out["bk"] = np.ascontiguousarray(bk.reshape(NPAIR, P).T).astype(np.float32)
    out["bv"] = _round_f32r(bv.reshape(1, D))
    out["bp"] = _round_f32r(np.asarray(b_proj, np.float32).reshape(1, D))
    return out


def host_prep_x(x_b, TOK, D):
    """One batch element [TOK, D] -> x^T tiled [P, KT*TOK]."""
    xT = np.ascontiguousarray(np.asarray(x_b, np.float32).T)  # [D, TOK]
    return _round_f32r(_tile_rows(xT))


# ---------------------------------------------------------------------------
# entry point
# ---------------------------------------------------------------------------

_BUILT = {}


def _get_nc(TOK, D, H, att_scale):
    key = (TOK, D, H, att_scale)
    if key not in _BUILT:
        nc = bacc.Bacc(
            "TRN2",
            target_bir_lowering=False,
            debug=False,
            dynamic_dma_scratch_size=512,
        )
        build(nc, TOK, D, H, att_scale)
        nc.compile()
        nc.finalize()
        _BUILT[key] = nc
    return _BUILT[key]


def kernel(x, w_qkv, b_qkv, w_proj, b_proj):
    from concourse.bass_utils import run_bass_kernel_spmd

    x = np.asarray(x, np.float32)
    B, TOK, D = x.shape
    H = H_FULL
    shared = host_prep_shared(
        np.asarray(w_qkv, np.float32),
        np.asarray(b_qkv, np.float32),
        np.asarray(w_proj, np.float32),
        np.asarray(b_proj, np.float32),
        D,
        H,
    )
    in_maps = []
    for b in range(B):
        m = dict(shared)
        m["x"] = host_prep_x(x[b], TOK, D)
        in_maps.append(m)

    nc = _get_nc(TOK, D, H, ATT_SCALE_FULL)
    res = run_bass_kernel_spmd(nc, in_maps, list(range(N_CORES)))
    out = np.stack([res.results[b]["out"] for b in range(B)], axis=0)
    return out.astype(np.float32)


# revision 9
# speedup vs baseline: 2.0330x; 1.2979x over previous
"""Multi-head attention kernel for Trainium2 (Bass/Tile), 8 NeuronCores.

Problem: nn_MultiHeadAttention
  x [8, 1024, 1024] f32, w_qkv [1024, 3072], b_qkv [3072],
  w_proj [1024, 1024], b_proj [1024]  ->  out [8, 1024, 1024]

  qkv = x @ w_qkv + b_qkv ; split (h, d, 3) interleaved on last dim
  score = q k^T per (b, h);  att = softmax(score, -1) / sqrt(1024)
  out = (att @ v) reshaped @ w_proj + b_proj

Sharding: data-parallel over batch. Each of the 8 cores runs the full
MHA for one batch element; no collectives.

v2 design (post-trace): the v1 kernel ran at 680us with the PE
clock-gated to 1.2GHz for 2/3 of the span (HAM re-throttle during
serial per-pair stalls) and 104us of single-partition DVE RECIPROCAL.
This version:
  - keeps the PE warm: attention is an ACT-paced conveyor (scores ->
    exp -> att@V per k-tile, ping-pong PSUM), with the NEXT pair's
    Q/K projections interleaved into the same span so the PE never
    idles long enough to re-throttle.
  - scores run as two concurrent row-tiled (64x128) matmuls: even
    head on PE tile T0 (SBUF rows 0:64), odd head on T8 (rows 64:128).
  - softmax denominators come free from a 65th "32.0" column in the
    V operand (folds the 1/sqrt(D) post-scale); their reciprocals are
    computed 2 rows at a time with reciprocal_approx_fast (~5x faster,
    128 partitions wide) and broadcast across partitions with one
    K=2 indicator matmul.
  - all weight DMA slices are contiguous per partition (per-pair /
    per-chunk major DRAM layout).

Device-side math per core (no on-device transpose anywhere):
  qT = (x wq)^T  [(h,d), tok]   kT likewise
  v_aug = [x wv + bv | 32.0] per head   [tok, h*(d+1)]
  per head pair, per k-tile: S^T[k,q] = kT.T-slice @ qT-slice (T0/T8)
     E = exp(S^T); O'^T[0:64,q] += v_aug.T @ E; O'^T[64,q] = 32*denom
  ao^T = O' * (1/(32*denom))  (recip via DVE approx, bcast via PE)
  out = ao^T.T @ wp + bp
"""

import os

os.environ.setdefault("MYCRO_LOCAL_CACHE", "1")

import numpy as np

import concourse.bass as bass
import concourse.tile as tile
from concourse import bacc, mybir

P = 128
DH = 64  # head dim
F32 = mybir.dt.float32
F32R = mybir.dt.float32r
# matmul-operand dtype: float32r streams at full PE rate when the
# moving free dim is >= 256
MM = F32R

# full-problem constants
B_FULL = 8
TOK_FULL = 1024
D_FULL = 1024
H_FULL = 16
ATT_SCALE_FULL = 1.0 / 32.0  # 1/sqrt(1024), applied after softmax
N_CORES = 8


def build(nc, TOK, D, H, att_scale):
    """Emit the one-core MHA program (one batch element).

    DRAM inputs (host pre-laid-out, all slices contiguous/partition):
      x        [P, KT*TOK]        [p][kt][t] = x[t, kt*P + p]
      wq, wk   [P, NPAIR*KT*P]    [p][pair][kt][n]; cols n = pair block
      wv, wp   [P, NVCH*KT*VCH]   [p][c][kt][n];  cols n = chunk block
      bq, bk   [P, NPAIR]         [p][pair] = b[pair*P + p]
      bv, bp   [1, D]
    Output: out [TOK, D] f32
    """
    assert D == H * DH and D % P == 0 and TOK % P == 0 and H % 2 == 0
    KT = D // P        # contraction tiles over the model dim
    MT = TOK // P      # token tiles (also the k-tiles of attention)
    NPAIR = H // 2     # head pairs
    VW = H * (DH + 1)  # v_aug row width: per head [v | aug]
    QCH = min(512, TOK)   # moving-chunk width for scores / att@V / QK
    NQH = TOK // QCH
    VCH = min(512, D)     # column chunk for V / proj weight streaming
    NVCH = D // VCH
    AUG = 1.0 / att_scale  # 32.0: folded post-softmax scale
    EXP = mybir.ActivationFunctionType.Exp
    assert MT % 2 == 0

    x_d = nc.dram_tensor("x", [P, KT * TOK], MM, kind="ExternalInput")
    wq_d = nc.dram_tensor("wq", [P, NPAIR * KT * P], MM, kind="ExternalInput")
    wk_d = nc.dram_tensor("wk", [P, NPAIR * KT * P], MM, kind="ExternalInput")
    wv_d = nc.dram_tensor("wv", [P, NVCH * KT * VCH], MM, kind="ExternalInput")
    wp_d = nc.dram_tensor("wp", [P, NVCH * KT * VCH], MM, kind="ExternalInput")
    bq_d = nc.dram_tensor("bq", [P, NPAIR], F32, kind="ExternalInput")
    bk_d = nc.dram_tensor("bk", [P, NPAIR], F32, kind="ExternalInput")
    bv_d = nc.dram_tensor("bv", [1, D], MM, kind="ExternalInput")
    bp_d = nc.dram_tensor("bp", [1, D], MM, kind="ExternalInput")
    out_d = nc.dram_tensor("out", [TOK, D], F32, kind="ExternalOutput")

    with tile.TileContext(nc) as tc:
        with (
            tc.tile_pool(name="sing", bufs=1) as sing,
            tc.tile_pool(name="psS", bufs=2, space="PSUM") as psS,
            tc.tile_pool(name="psO", bufs=4, space="PSUM") as psO,
            tc.tile_pool(name="ebuf", bufs=4) as ebuf,
            tc.tile_pool(name="qkp", bufs=2) as qkp,
            tc.tile_pool(name="wqkp", bufs=2) as wqkp,
            tc.tile_pool(name="rbuf", bufs=2) as rbuf,
            tc.tile_pool(name="outp", bufs=2) as outp,
        ):
            # ---------------- persistent SBUF ----------------
            # memset cannot target f32r; stage in f32, round via DVE copy
            cst_sb = sing.tile([2, P], F32, tag="cst")
            nc.vector.memset(cst_sb, 1.0)
            ones_sb = sing.tile([1, P], MM, tag="ones")
            nc.vector.tensor_copy(out=ones_sb, in_=cst_sb[0:1, :])
            vones_sb = sing.tile([P, MT * H], F32, tag="vones")
            nc.vector.memset(vones_sb, AUG)

            bq_sb = sing.tile([P, NPAIR], F32, tag="bq")
            nc.sync.dma_start(out=bq_sb, in_=bq_d[:, :])
            bk_sb = sing.tile([P, NPAIR], F32, tag="bk")
            nc.sync.dma_start(out=bk_sb, in_=bk_d[:, :])
            bv_sb = sing.tile([1, D], MM, tag="bv")
            nc.sync.dma_start(out=bv_sb, in_=bv_d[:, :])
            bp_sb = sing.tile([1, D], MM, tag="bp")
            nc.sync.dma_start(out=bp_sb, in_=bp_d[:, :])

            v_sb = sing.tile([P, MT, VW], MM, tag="v")   # v_aug
            # aug columns (denominator accumulators) = 1/att_scale
            nc.vector.tensor_copy(
                out=v_sb[:, :, :]
                .rearrange("p m (h e) -> p m h e", e=DH + 1)[:, :, :, DH],
                in_=vones_sb[:, :].rearrange("p (m h) -> p m h", h=H),
            )
            ao_sb = sing.tile([P, NPAIR, TOK], MM, tag="ao")  # attout^T

            x_sb = sing.tile([P, KT, TOK], MM, tag="x")
            half = KT // 2
            nc.sync.dma_start(
                out=x_sb[:, 0:half, :], in_=x_d[:, 0 : half * TOK]
            )
            nc.sync.dma_start(
                out=x_sb[:, half:KT, :], in_=x_d[:, half * TOK : KT * TOK]
            )
            x3 = x_sb

            wv3 = wv_d[:, :].rearrange("p (c kt n) -> p c kt n", c=NVCH, kt=KT)
            wp3 = wp_d[:, :].rearrange("p (c kt n) -> p c kt n", c=NVCH, kt=KT)
            wq3 = wq_d[:, :].rearrange("p (pr kt n) -> p pr kt n", pr=NPAIR, kt=KT)
            wk3 = wk_d[:, :].rearrange("p (pr kt n) -> p pr kt n", pr=NPAIR, kt=KT)

            # ---------------- Q/K projection task ----------------
            # qT/kT for one pair: [P rows = (even|odd head dims), TOK]
            qT = {}
            kT = {}

            def load_wqk(p):
                wq_sb = wqkp.tile([P, KT, P], MM, tag="wq")
                nc.sync.dma_start(
                    out=wq_sb, in_=wq3[:, p, :, :]
                )
                wk_sb = wqkp.tile([P, KT, P], MM, tag="wk")
                nc.sync.dma_start(
                    out=wk_sb, in_=wk3[:, p, :, :]
                )
                return wq_sb, wk_sb

            def emit_qk_task(p, which, w_sb, c0, cw):
                """One accumulation task: (x @ w_pair)^T chunk + bias."""
                if which == "q":
                    if p not in qT:
                        qT[p] = qkp.tile([P, TOK], MM, tag="qT", name=f"qT{p}")
                    dst, b_sb = qT[p], bq_sb
                else:
                    if p not in kT:
                        kT[p] = qkp.tile([P, TOK], MM, tag="kT", name=f"kT{p}")
                    dst, b_sb = kT[p], bk_sb
                ps = psO.tile([P, QCH], F32, tag="O")
                for kt in range(KT):
                    nc.tensor.matmul(
                        ps[:, 0:cw],
                        lhsT=w_sb[:, kt, :],
                        rhs=x3[:, kt, c0 : c0 + cw],
                        start=(kt == 0),
                        stop=(kt == KT - 1),
                    )
                nc.vector.tensor_scalar_add(
                    out=dst[:, c0 : c0 + cw],
                    in0=ps[:, 0:cw],
                    scalar1=b_sb[:, p : p + 1],
                )

            # ---------------- V phase: v_aug = [x wv + bv | AUG] --------
            wqk0 = load_wqk(0)
            with tc.tile_pool(name="wvp", bufs=2) as wvp:
                for c in range(NVCH):
                    wv_sb = wvp.tile([P, KT, VCH], MM, tag="wv")
                    nc.sync.dma_start(out=wv_sb, in_=wv3[:, c, :, :])
                    for mt in range(MT):
                        ps_v = psO.tile([P, QCH], F32, tag="O")
                        for kt in range(KT):
                            nc.tensor.matmul(
                                ps_v[:, 0:VCH],
                                lhsT=x3[:, kt, mt * P : (mt + 1) * P],
                                rhs=wv_sb[:, kt, :],
                                start=(kt == 0),
                                stop=False,
                            )
                        nc.tensor.matmul(
                            ps_v[:, 0:VCH],
                            lhsT=ones_sb[0:1, 0:P],
                            rhs=bv_sb[0:1, c * VCH : (c + 1) * VCH],
                            start=False,
                            stop=True,
                        )
                        # scatter heads into v_aug (DH+1 stride)
                        nh = VCH // DH
                        h0 = c * VCH // DH
                        nc.vector.tensor_copy(
                            out=v_sb[:, mt, :]
                            .rearrange("p (h e) -> p h e", e=DH + 1)[
                                :, h0 : h0 + nh, 0:DH
                            ],
                            in_=ps_v[:, 0:VCH].rearrange(
                                "p (h d) -> p h d", d=DH
                            ),
                        )
                    # interleave pair-0 Q/K projections into the V phase
                    if c == 0:
                        for c0 in range(0, TOK, QCH):
                            emit_qk_task(0, "q", wqk0[0], c0, QCH)
                    if c == NVCH - 1 or NVCH == 1:
                        for c0 in range(0, TOK, QCH):
                            emit_qk_task(0, "k", wqk0[1], c0, QCH)

            # ---------------- attention, per head pair ----------------
            # Software-pipelined conveyor: per 2-kb group emit scores,
            # then exp, then the PREVIOUS group's att@V (so the PE queue
            # never sits behind an exp that hasn't finished). The
            # normalize for each (pair, qh) is split: the DVE reciprocal
            # chain is emitted at block end, but the PE broadcast + final
            # muls are deferred into the next block.
            pending_norm = [None]

            def flush_norm():
                if pending_norm[0] is None:
                    return
                p_, q0_, ps_oe_, ps_oo_, rc_ = pending_norm[0]
                pending_norm[0] = None
                # partition-broadcast via two K=1 ones-column matmuls
                # (dst base partition must be 0; use column ranges)
                bc = psS.tile([P, 2 * QCH], F32, tag="S")
                nc.tensor.matmul(
                    bc[0:DH, 0:QCH],
                    lhsT=ones_sb[0:1, 0:DH],
                    rhs=rc_[0:1, 0:QCH],
                    start=True,
                    stop=True,
                )
                nc.tensor.matmul(
                    bc[0:DH, QCH : 2 * QCH],
                    lhsT=ones_sb[0:1, 0:DH],
                    rhs=rc_[0:1, QCH : 2 * QCH],
                    start=True,
                    stop=True,
                )
                bcs = rbuf.tile([P, 2 * QCH], MM, tag="bcs")
                nc.vector.tensor_copy(out=bcs[0:DH, :], in_=bc[0:DH, :])
                nc.vector.tensor_mul(
                    out=ao_sb[0:DH, p_, q0_ : q0_ + QCH],
                    in0=ps_oe_[0:DH, :],
                    in1=bcs[0:DH, 0:QCH],
                )
                nc.vector.tensor_mul(
                    out=ao_sb[DH:P, p_, q0_ : q0_ + QCH],
                    in0=ps_oo_[0:DH, :],
                    in1=bcs[0:DH, QCH : 2 * QCH],
                )

            for p in range(NPAIR):
                wqk_next = load_wqk(p + 1) if p + 1 < NPAIR else None
                for qh in range(NQH):
                    q0 = qh * QCH
                    ps_oe = psO.tile([P, QCH], F32, tag="O", name="ps_oe")
                    ps_oo = psO.tile([P, QCH], F32, tag="O", name="ps_oo")

                    def emit_attv(kbs, e_ts):
                        for kb, et in zip(kbs, e_ts):
                            for hoff, ps_out in ((0, ps_oe), (1, ps_oo)):
                                hh = 2 * p + hoff
                                nc.tensor.matmul(
                                    ps_out[0 : DH + 1, :],
                                    lhsT=v_sb[
                                        :, kb,
                                        hh * (DH + 1) : (hh + 1) * (DH + 1),
                                    ],
                                    rhs=et[:, hoff * QCH : (hoff + 1) * QCH],
                                    start=(kb == 0),
                                    stop=(kb == MT - 1),
                                    skip_group_check=True,
                                )

                    prev = None
                    for g in range(0, MT, 2):
                        kbs = (g, g + 1)
                        e_ts = []
                        for kb in kbs:
                            st = psS.tile([P, 2 * QCH], F32, tag="S")
                            # even head: PE tile T0 (SBUF rows 0:64)
                            # odd head: T8 (rows 64:128) -- concurrent
                            nc.tensor.matmul(
                                st[:, 0:QCH],
                                lhsT=kT[p][0:DH, kb * P : (kb + 1) * P],
                                rhs=qT[p][0:DH, q0 : q0 + QCH],
                                start=True,
                                stop=True,
                            )
                            nc.tensor.matmul(
                                st[:, QCH : 2 * QCH],
                                lhsT=kT[p][DH:P, kb * P : (kb + 1) * P],
                                rhs=qT[p][DH:P, q0 : q0 + QCH],
                                start=True,
                                stop=True,
                            )
                            et = ebuf.tile([P, 2 * QCH], MM, tag="E")
                            nc.scalar.activation(out=et, in_=st, func=EXP)
                            e_ts.append(et)
                        if g == 2:
                            # previous block's deferred PE broadcast+muls
                            flush_norm()
                        if prev is not None:
                            emit_attv(*prev)
                        # next pair's Q (during qh 0) / K (during the last
                        # qh) projection tasks, before the last attV flush
                        if wqk_next is not None and g + 2 >= MT:
                            tasks = []
                            if qh == 0:
                                tasks.append(("q", wqk_next[0]))
                            if qh == NQH - 1:
                                tasks.append(("k", wqk_next[1]))
                            for which, w_sb in tasks:
                                for c0 in range(0, TOK, QCH):
                                    emit_qk_task(p + 1, which, w_sb, c0, QCH)
                        prev = (kbs, e_ts)
                    emit_attv(*prev)

                    # ---- normalize DVE chain: 1/(32*denom) ----
                    rq = rbuf.tile([1, 2 * QCH], F32, tag="rq")
                    nc.vector.tensor_copy(
                        out=rq[0:1, 0:QCH], in_=ps_oe[DH : DH + 1, :]
                    )
                    nc.vector.tensor_copy(
                        out=rq[0:1, QCH : 2 * QCH],
                        in_=ps_oo[DH : DH + 1, :],
                    )
                    rr = rbuf.tile([1, 2 * QCH], F32, tag="rr")
                    nc.vector.reciprocal_approx_fast(out=rr, in_=rq)
                    rc = rbuf.tile([1, 2 * QCH], MM, tag="rc")
                    nc.vector.tensor_copy(out=rc, in_=rr)
                    flush_norm()  # no-op unless MT==2 left it pending
                    pending_norm[0] = (p, q0, ps_oe, ps_oo, rc)
            flush_norm()

            # ---------------- projection: out = ao^T.T wp + bp -------
            with tc.tile_pool(name="wpp", bufs=2) as wpp:
                for c in range(NVCH):
                    wp_sb = wpp.tile([P, KT, VCH], MM, tag="wp")
                    nc.sync.dma_start(out=wp_sb, in_=wp3[:, c, :, :])
                    for mt in range(MT):
                        ps_p = psO.tile([P, QCH], F32, tag="O")
                        for kt in range(KT):
                            nc.tensor.matmul(
                                ps_p[:, 0:VCH],
                                lhsT=ao_sb[:, kt, mt * P : (mt + 1) * P],
                                rhs=wp_sb[:, kt, :],
                                start=(kt == 0),
                                stop=False,
                            )
                        nc.tensor.matmul(
                            ps_p[:, 0:VCH],
                            lhsT=ones_sb[0:1, 0:P],
                            rhs=bp_sb[0:1, c * VCH : (c + 1) * VCH],
                            start=False,
                            stop=True,
                        )
                        o_sb = outp.tile([P, VCH], F32, tag="o")
                        nc.vector.tensor_copy(
                            out=o_sb[:, 0:VCH], in_=ps_p[:, 0:VCH]
                        )
                        nc.sync.dma_start(
                            out=out_d[
                                mt * P : (mt + 1) * P,
                                c * VCH : (c + 1) * VCH,
                            ],
                            in_=o_sb[:, 0:VCH],
                        )

    return nc


# ---------------------------------------------------------------------------
# host-side layout prep
# ---------------------------------------------------------------------------

def _round_f32r(x):
    """RNE to f32r's 11-explicit-mantissa-bit grid (matches HW rounding)."""
    u = np.ascontiguousarray(x, np.float32).view(np.uint32)
    u = ((u + np.uint32(1 << 11)) >> 12) << 12
    return u.view(np.float32)


def _tile_rows(w):
    """[D, N] -> [P, (D//P) * N] with [p][kt][n] layout."""
    Dd, N = w.shape
    KT = Dd // P
    return np.ascontiguousarray(
        w.reshape(KT, P, N).transpose(1, 0, 2).reshape(P, KT * N)
    )


def host_prep_shared(w_qkv, b_qkv, w_proj, b_proj, D, H):
    """Split/retile the weights once for all cores."""
    NPAIR = H // 2
    VCH = min(512, D)
    NVCH = D // VCH

    wq3 = w_qkv.reshape(D, H, DH, 3)
    wq = np.ascontiguousarray(wq3[:, :, :, 0].reshape(D, D))
    wk = np.ascontiguousarray(wq3[:, :, :, 1].reshape(D, D))
    wv = np.ascontiguousarray(wq3[:, :, :, 2].reshape(D, D))
    wp = np.ascontiguousarray(np.asarray(w_proj, np.float32))

    def pair_major(w):  # [D, D] -> [P, NPAIR*KT*P], pair-block major
        blocks = [
            _tile_rows(w[:, p * P : (p + 1) * P]) for p in range(NPAIR)
        ]
        return _round_f32r(np.concatenate(blocks, axis=1))

    def chunk_major(w):  # [D, D] -> [P, NVCH*KT*VCH], chunk major
        blocks = [
            _tile_rows(w[:, c * VCH : (c + 1) * VCH]) for c in range(NVCH)
        ]
        return _round_f32r(np.concatenate(blocks, axis=1))

    out = {
        "wq": pair_major(wq),
        "wk": pair_major(wk),
        "wv": chunk_major(wv),
        "wp": chunk_major(wp),
    }
    b3 = np.asarray(b_qkv, np.float32).reshape(H, DH, 3)
    bq = np.ascontiguousarray(b3[:, :, 0].reshape(D))
    bk = np.ascontiguousarray(b3[:, :, 1].reshape(D))
    bv = np.ascontiguousarray(b3[:, :, 2].reshape(D))
    out["bq"] = np.ascontiguousarray(bq.reshape(NPAIR, P).T).astype(np.float32)
    out["bk"] = np.ascontiguousarray(bk.reshape(NPAIR, P).T).astype(np.float32)
    out["bv"] = _round_f32r(bv.reshape(1, D))
    out["bp"] = _round_f32r(np.asarray(b_proj, np.float32).reshape(1, D))
    return out


def host_prep_x(x_b, TOK, D):
    """One batch element [TOK, D] -> x^T tiled [P, KT*TOK]."""
    xT = np.ascontiguousarray(np.asarray(x_b, np.float32).T)  # [D, TOK]
    return _round_f32r(_tile_rows(xT))


# ---------------------------------------------------------------------------
# entry point
# ---------------------------------------------------------------------------

_BUILT = {}


def _get_nc(TOK, D, H, att_scale):
    key = (TOK, D, H, att_scale)
    if key not in _BUILT:
        nc = bacc.Bacc(
            "TRN2",
            target_bir_lowering=False,
            debug=False,
            dynamic_dma_scratch_size=512,
        )
        build(nc, TOK, D, H, att_scale)
        nc.compile()
        nc.finalize()
        _BUILT[key] = nc
    return _BUILT[key]


def kernel(x, w_qkv, b_qkv, w_proj, b_proj):
    from concourse.bass_utils import run_bass_kernel_spmd

    x = np.asarray(x, np.float32)
    B, TOK, D = x.shape
    H = H_FULL
    shared = host_prep_shared(
        np.asarray(w_qkv, np.float32),
        np.asarray(b_qkv, np.float32),
        np.asarray(w_proj, np.float32),
        np.asarray(b_proj, np.float32),
        D,
        H,
    )
    in_maps = []
    for b in range(B):
        m = dict(shared)
        m["x"] = host_prep_x(x[b], TOK, D)
        in_maps.append(m)

    nc = _get_nc(TOK, D, H, ATT_SCALE_FULL)
    res = run_bass_kernel_spmd(nc, in_maps, list(range(N_CORES)))
    out = np.stack([res.results[b]["out"] for b in range(B)], axis=0)
    return out.astype(np.float32)


# revision 18
# speedup vs baseline: 2.1876x; 1.0760x over previous
"""Multi-head attention kernel for Trainium2 (Bass/Tile), 8 NeuronCores.

Problem: nn_MultiHeadAttention
  x [8, 1024, 1024] f32, w_qkv [1024, 3072], b_qkv [3072],
  w_proj [1024, 1024], b_proj [1024]  ->  out [8, 1024, 1024]

  qkv = x @ w_qkv + b_qkv ; split (h, d, 3) interleaved on last dim
  score = q k^T per (b, h);  att = softmax(score, -1) / sqrt(1024)
  out = (att @ v) reshaped @ w_proj + b_proj

Sharding: data-parallel over batch. Each of the 8 cores runs the full
MHA for one batch element; no collectives.

v2 design (post-trace): the v1 kernel ran at 680us with the PE
clock-gated to 1.2GHz for 2/3 of the span (HAM re-throttle during
serial per-pair stalls) and 104us of single-partition DVE RECIPROCAL.
This version:
  - keeps the PE warm: attention is an ACT-paced conveyor (scores ->
    exp -> att@V per k-tile, ping-pong PSUM), with the NEXT pair's
    Q/K projections interleaved into the same span so the PE never
    idles long enough to re-throttle.
  - scores run as two concurrent row-tiled (64x128) matmuls: even
    head on PE tile T0 (SBUF rows 0:64), odd head on T8 (rows 64:128).
  - softmax denominators come free from a 65th "32.0" column in the
    V operand (folds the 1/sqrt(D) post-scale); their reciprocals are
    computed 2 rows at a time with reciprocal_approx_fast (~5x faster,
    128 partitions wide) and broadcast across partitions with one
    K=2 indicator matmul.
  - all weight DMA slices are contiguous per partition (per-pair /
    per-chunk major DRAM layout).

Device-side math per core (no on-device transpose anywhere):
  qT = (x wq)^T  [(h,d), tok]   kT likewise
  v_aug = [x wv + bv | 32.0] per head   [tok, h*(d+1)]
  per head pair, per k-tile: S^T[k,q] = kT.T-slice @ qT-slice (T0/T8)
     E = exp(S^T); O'^T[0:64,q] += v_aug.T @ E; O'^T[64,q] = 32*denom
  ao^T = O' * (1/(32*denom))  (recip via DVE approx, bcast via PE)
  out = ao^T.T @ wp + bp
"""

import os

os.environ.setdefault("MYCRO_LOCAL_CACHE", "1")

import numpy as np

import concourse.bass as bass
import concourse.tile as tile
from concourse import bacc, mybir

P = 128
DH = 64  # head dim
F32 = mybir.dt.float32
F32R = mybir.dt.float32r
BF = mybir.dt.bfloat16
# matmul-operand dtype for the score path (x, wq/wk, qT/kT): f32r keeps
# 11 mantissa bits, needed because score errors pass through exp().
# Everything else (V, E, attout, proj) runs bf16: same 1 cycle/col PE
# rate but fast weight loads and half the DMA/SBUF footprint.
MM = F32R

# full-problem constants
B_FULL = 8
TOK_FULL = 1024
D_FULL = 1024
H_FULL = 16
ATT_SCALE_FULL = 1.0 / 32.0  # 1/sqrt(1024), applied after softmax
N_CORES = 8


def build(nc, TOK, D, H, att_scale):
    """Emit the one-core MHA program (one batch element).

    DRAM inputs (host pre-laid-out, all slices contiguous/partition):
      x        [P, KT*TOK]        [p][kt][t] = x[t, kt*P + p]
      wq, wk   [P, NPAIR*KT*P]    [p][pair][kt][n]; cols n = pair block
      wv, wp   [P, NVCH*KT*VCH]   [p][c][kt][n];  cols n = chunk block
      bq, bk   [P, NPAIR]         [p][pair] = b[pair*P + p]
      bv, bp   [1, D]
    Output: out [TOK, D] f32
    """
    assert D == H * DH and D % P == 0 and TOK % P == 0 and H % 2 == 0
    KT = D // P        # contraction tiles over the model dim
    MT = TOK // P      # token tiles (also the k-tiles of attention)
    NPAIR = H // 2     # head pairs
    VW = H * (DH + 1)  # v_aug row width: per head [v | aug]
    QCH = min(512, TOK)   # moving-chunk width for scores / att@V / QK
    NQH = TOK // QCH
    VCH = min(512, D)     # column chunk for V / proj weight streaming
    NVCH = D // VCH
    AUG = 1.0 / att_scale  # 32.0: folded post-softmax scale
    EXP = mybir.ActivationFunctionType.Exp
    assert MT % 2 == 0

    x_d = nc.dram_tensor("x", [P, KT * TOK], MM, kind="ExternalInput")
    xv_d = nc.dram_tensor("xv", [P, KT * TOK], BF, kind="ExternalInput")
    wq_d = nc.dram_tensor("wq", [P, NPAIR * KT * P], MM, kind="ExternalInput")
    wk_d = nc.dram_tensor("wk", [P, NPAIR * KT * P], MM, kind="ExternalInput")
    wv_d = nc.dram_tensor("wv", [P, NVCH * KT * VCH], BF, kind="ExternalInput")
    wp_d = nc.dram_tensor("wp", [P, NVCH * KT * VCH], BF, kind="ExternalInput")
    bq_d = nc.dram_tensor("bq", [P, NPAIR], F32, kind="ExternalInput")
    bk_d = nc.dram_tensor("bk", [P, NPAIR], F32, kind="ExternalInput")
    bv_d = nc.dram_tensor("bv", [1, D], BF, kind="ExternalInput")
    bp_d = nc.dram_tensor("bp", [1, D], BF, kind="ExternalInput")
    out_d = nc.dram_tensor("out", [TOK, D], F32, kind="ExternalOutput")

    with tile.TileContext(nc) as tc:
        with (
            tc.tile_pool(name="sing", bufs=1) as sing,
            tc.tile_pool(name="psS", bufs=2, space="PSUM") as psS,
            tc.tile_pool(name="psO", bufs=4, space="PSUM") as psO,
            tc.tile_pool(name="ebuf", bufs=4) as ebuf,
            tc.tile_pool(name="qkp", bufs=2) as qkp,
            tc.tile_pool(name="wqkp", bufs=2) as wqkp,
            tc.tile_pool(name="rbuf", bufs=2) as rbuf,
            tc.tile_pool(name="outp", bufs=2) as outp,
        ):
            from concourse import library_config

            nc.gpsimd.load_library(library_config.attn)

            # ---------------- persistent SBUF ----------------
            # DMA priority order: bf16 V-phase inputs first (smallest
            # path to first matmul), then the f32r score-path inputs.
            xv_sb = sing.tile([P, KT, TOK], BF, tag="xv")
            nc.sync.dma_start(out=xv_sb, in_=xv_d[:, :])

            # memset cannot target f32r/bf16; stage in f32, cast via DVE
            cst_sb = sing.tile([2, P], F32, tag="cst")
            nc.vector.memset(cst_sb, 1.0)
            ones_bf = sing.tile([1, P], BF, tag="ones")
            nc.vector.tensor_copy(out=ones_bf, in_=cst_sb[0:1, :])
            vones_sb = sing.tile([P, MT * H], F32, tag="vones")
            nc.vector.memset(vones_sb, AUG)

            x_sb = sing.tile([P, KT, TOK], MM, tag="x")
            half = KT // 2
            nc.sync.dma_start(
                out=x_sb[:, 0:half, :], in_=x_d[:, 0 : half * TOK]
            )
            nc.sync.dma_start(
                out=x_sb[:, half:KT, :], in_=x_d[:, half * TOK : KT * TOK]
            )
            x3 = x_sb

            bq_sb = sing.tile([P, NPAIR], F32, tag="bq")
            nc.sync.dma_start(out=bq_sb, in_=bq_d[:, :])
            bk_sb = sing.tile([P, NPAIR], F32, tag="bk")
            nc.sync.dma_start(out=bk_sb, in_=bk_d[:, :])
            bv_sb = sing.tile([1, D], BF, tag="bv")
            nc.sync.dma_start(out=bv_sb, in_=bv_d[:, :])
            bp_sb = sing.tile([1, D], BF, tag="bp")
            nc.sync.dma_start(out=bp_sb, in_=bp_d[:, :])

            v_sb = sing.tile([P, MT, VW], BF, tag="v")   # v_aug
            # aug columns (denominator accumulators) = 1/att_scale
            nc.vector.tensor_copy(
                out=v_sb[:, :, :]
                .rearrange("p m (h e) -> p m h e", e=DH + 1)[:, :, :, DH],
                in_=vones_sb[:, :].rearrange("p (m h) -> p m h", h=H),
            )
            ao_sb = sing.tile([P, NPAIR, TOK], BF, tag="ao")  # attout^T

            wv3 = wv_d[:, :].rearrange("p (c kt n) -> p c kt n", c=NVCH, kt=KT)
            wp3 = wp_d[:, :].rearrange("p (c kt n) -> p c kt n", c=NVCH, kt=KT)
            wq3 = wq_d[:, :].rearrange("p (pr kt n) -> p pr kt n", pr=NPAIR, kt=KT)
            wk3 = wk_d[:, :].rearrange("p (pr kt n) -> p pr kt n", pr=NPAIR, kt=KT)

            # ---------------- Q/K projection task ----------------
            # qT/kT for one pair: [P rows = (even|odd head dims), TOK]
            qT = {}
            kT = {}

            def load_wqk(p):
                wq_sb = wqkp.tile([P, KT, P], MM, tag="wq")
                nc.sync.dma_start(
                    out=wq_sb, in_=wq3[:, p, :, :]
                )
                wk_sb = wqkp.tile([P, KT, P], MM, tag="wk")
                nc.sync.dma_start(
                    out=wk_sb, in_=wk3[:, p, :, :]
                )
                return wq_sb, wk_sb

            def emit_qk_task(p, which, w_sb, c0, cw):
                """One accumulation task: (x @ w_pair)^T chunk + bias."""
                if which == "q":
                    if p not in qT:
                        qT[p] = qkp.tile([P, TOK], MM, tag="qT", name=f"qT{p}")
                    dst, b_sb = qT[p], bq_sb
                else:
                    if p not in kT:
                        kT[p] = qkp.tile([P, TOK], MM, tag="kT", name=f"kT{p}")
                    dst, b_sb = kT[p], bk_sb
                ps = psO.tile([P, QCH], F32, tag="O")
                for kt in range(KT):
                    nc.tensor.matmul(
                        ps[:, 0:cw],
                        lhsT=w_sb[:, kt, :],
                        rhs=x3[:, kt, c0 : c0 + cw],
                        start=(kt == 0),
                        stop=(kt == KT - 1),
                    )
                nc.vector.tensor_scalar_add(
                    out=dst[:, c0 : c0 + cw],
                    in0=ps[:, 0:cw],
                    scalar1=b_sb[:, p : p + 1],
                )

            # ---------------- V phase: v_aug = [x wv + bv | AUG] --------
            wv_sb = sing.tile([P, NVCH, KT, VCH], BF, tag="wv")
            nc.sync.dma_start(out=wv_sb, in_=wv3[:, :, :, :])
            wp_sb = sing.tile([P, NVCH, KT, VCH], BF, tag="wp")
            nc.sync.dma_start(out=wp_sb, in_=wp3[:, :, :, :])
            wqk0 = load_wqk(0)
            for c in range(NVCH):
                for mt in range(MT):
                    ps_v = psO.tile([P, QCH], F32, tag="O")
                    for kt in range(KT):
                        nc.tensor.matmul(
                            ps_v[:, 0:VCH],
                            lhsT=xv_sb[:, kt, mt * P : (mt + 1) * P],
                            rhs=wv_sb[:, c, kt, :],
                            start=(kt == 0),
                            stop=False,
                        )
                    nc.tensor.matmul(
                        ps_v[:, 0:VCH],
                        lhsT=ones_bf[0:1, 0:P],
                        rhs=bv_sb[0:1, c * VCH : (c + 1) * VCH],
                        start=False,
                        stop=True,
                    )
                    # scatter heads into v_aug (DH+1 stride)
                    nh = VCH // DH
                    h0 = c * VCH // DH
                    nc.vector.tensor_copy(
                        out=v_sb[:, mt, :]
                        .rearrange("p (h e) -> p h e", e=DH + 1)[
                            :, h0 : h0 + nh, 0:DH
                        ],
                        in_=ps_v[:, 0:VCH].rearrange(
                            "p (h d) -> p h d", d=DH
                        ),
                    )
                # interleave pair-0 Q/K projections into the V phase
                if c == 0:
                    for c0 in range(0, TOK, QCH):
                        emit_qk_task(0, "q", wqk0[0], c0, QCH)
                if c == NVCH - 1 or NVCH == 1:
                    for c0 in range(0, TOK, QCH):
                        emit_qk_task(0, "k", wqk0[1], c0, QCH)

            # ---------------- attention, per head pair ----------------
            # Software-pipelined conveyor: per 2-kb group emit scores,
            # then exp, then the PREVIOUS group's att@V (so the PE queue
            # never sits behind an exp that hasn't finished). The
            # normalize for each (pair, qh) is split: the DVE reciprocal
            # chain is emitted at block end, but the PE broadcast + final
            # muls are deferred into the next block.
            pending_norm = [None]

            def flush_norm():
                if pending_norm[0] is None:
                    return
                p_, q0_, ps_oe_, ps_oo_, rc_ = pending_norm[0]
                pending_norm[0] = None
                # partition-broadcast of both heads' reciprocals (GPSIMD,
                # keeps the PE out of the normalize chain entirely)
                bcs = rbuf.tile([DH, 2 * QCH], BF, tag="bcs")
                nc.gpsimd.partition_broadcast(
                    out_ap=bcs, in_ap=rc_[0:1, :], channels=DH
                )
                nc.vector.tensor_mul(
                    out=ao_sb[0:DH, p_, q0_ : q0_ + QCH],
                    in0=ps_oe_[0:DH, :],
                    in1=bcs[0:DH, 0:QCH],
                )
                nc.vector.tensor_mul(
                    out=ao_sb[DH:P, p_, q0_ : q0_ + QCH],
                    in0=ps_oo_[0:DH, :],
                    in1=bcs[0:DH, QCH : 2 * QCH],
                )

            for p in range(NPAIR):
                wqk_next = load_wqk(p + 1) if p + 1 < NPAIR else None
                for qh in range(NQH):
                    q0 = qh * QCH
                    ps_oe = psO.tile([P, QCH], F32, tag="O", name="ps_oe")
                    ps_oo = psO.tile([P, QCH], F32, tag="O", name="ps_oo")

                    def emit_attv(kbs, e_ts):
                        for kb, et in zip(kbs, e_ts):
                            for hoff, ps_out in ((0, ps_oe), (1, ps_oo)):
                                hh = 2 * p + hoff
                                nc.tensor.matmul(
                                    ps_out[0 : DH + 1, :],
                                    lhsT=v_sb[
                                        :, kb,
                                        hh * (DH + 1) : (hh + 1) * (DH + 1),
                                    ],
                                    rhs=et[:, hoff * QCH : (hoff + 1) * QCH],
                                    start=(kb == 0),
                                    stop=(kb == MT - 1),
                                    skip_group_check=True,
                                )

                    prev = None
                    for g in range(0, MT, 2):
                        kbs = (g, g + 1)
                        e_ts = []
                        for kb in kbs:
                            st = psS.tile([P, 2 * QCH], F32, tag="S")
                            # even head: PE tile T0 (SBUF rows 0:64)
                            # odd head: T8 (rows 64:128) -- concurrent
                            nc.tensor.matmul(
                                st[:, 0:QCH],
                                lhsT=kT[p][0:DH, kb * P : (kb + 1) * P],
                                rhs=qT[p][0:DH, q0 : q0 + QCH],
                                start=True,
                                stop=True,
                            )
                            nc.tensor.matmul(
                                st[:, QCH : 2 * QCH],
                                lhsT=kT[p][DH:P, kb * P : (kb + 1) * P],
                                rhs=qT[p][DH:P, q0 : q0 + QCH],
                                start=True,
                                stop=True,
                            )
                            et = ebuf.tile([P, 2 * QCH], BF, tag="E")
                            nc.scalar.activation(out=et, in_=st, func=EXP)
                            e_ts.append(et)
                        if g == 2:
                            # previous block's deferred PE broadcast+muls
                            flush_norm()
                        if prev is not None:
                            emit_attv(*prev)
                        # next pair's Q (during qh 0) / K (during the last
                        # qh) projection tasks, before the last attV flush
                        if wqk_next is not None and g + 2 >= MT:
                            tasks = []
                            if qh == 0:
                                tasks.append(("q", wqk_next[0]))
                            if qh == NQH - 1:
                                tasks.append(("k", wqk_next[1]))
                            for which, w_sb in tasks:
                                for c0 in range(0, TOK, QCH):
                                    emit_qk_task(p + 1, which, w_sb, c0, QCH)
                        prev = (kbs, e_ts)
                    emit_attv(*prev)

                    # ---- normalize DVE chain: 1/(32*denom) ----
                    rq = rbuf.tile([1, 2 * QCH], F32, tag="rq")
                    nc.vector.tensor_copy(
                        out=rq[0:1, 0:QCH], in_=ps_oe[DH : DH + 1, :]
                    )
                    nc.vector.tensor_copy(
                        out=rq[0:1, QCH : 2 * QCH],
                        in_=ps_oo[DH : DH + 1, :],
                    )
                    rr = rbuf.tile([1, 2 * QCH], F32, tag="rr")
                    nc.vector.reciprocal_approx_fast(out=rr, in_=rq)
                    rc = rbuf.tile([1, 2 * QCH], BF, tag="rc")
                    nc.vector.tensor_copy(out=rc, in_=rr)
                    flush_norm()  # no-op unless MT==2 left it pending
                    pending_norm[0] = (p, q0, ps_oe, ps_oo, rc)
            flush_norm()

            # ---------------- projection: out = ao^T.T wp + bp -------
            # kt order is pair-completion order, so the scheduler can
            # run the first KT-1 accumulation steps of each output tile
            # during the last pair's attention.
            for c in range(NVCH):
                for mt in range(MT):
                    ps_p = psO.tile([P, QCH], F32, tag="O")
                    for kt in range(KT):
                        nc.tensor.matmul(
                            ps_p[:, 0:VCH],
                            lhsT=ao_sb[:, kt, mt * P : (mt + 1) * P],
                            rhs=wp_sb[:, c, kt, :],
                            start=(kt == 0),
                            stop=False,
                        )
                    nc.tensor.matmul(
                        ps_p[:, 0:VCH],
                        lhsT=ones_bf[0:1, 0:P],
                        rhs=bp_sb[0:1, c * VCH : (c + 1) * VCH],
                        start=False,
                        stop=True,
                    )
                    o_sb = outp.tile([P, VCH], F32, tag="o")
                    nc.vector.tensor_copy(
                        out=o_sb[:, 0:VCH], in_=ps_p[:, 0:VCH]
                    )
                    nc.sync.dma_start(
                        out=out_d[
                            mt * P : (mt + 1) * P,
                            c * VCH : (c + 1) * VCH,
                        ],
                        in_=o_sb[:, 0:VCH],
                    )

    return nc


# ---------------------------------------------------------------------------
# host-side layout prep
# ---------------------------------------------------------------------------

def _round_f32r(x):
    """RNE to f32r's 11-explicit-mantissa-bit grid (matches HW rounding)."""
    u = np.ascontiguousarray(x, np.float32).view(np.uint32)
    u = ((u + np.uint32(1 << 11)) >> 12) << 12
    return u.view(np.float32)


def _tile_rows(w):
    """[D, N] -> [P, (D//P) * N] with [p][kt][n] layout."""
    Dd, N = w.shape
    KT = Dd // P
    return np.ascontiguousarray(
        w.reshape(KT, P, N).transpose(1, 0, 2).reshape(P, KT * N)
    )


def host_prep_shared(w_qkv, b_qkv, w_proj, b_proj, D, H):
    """Split/retile the weights once for all cores."""
    NPAIR = H // 2
    VCH = min(512, D)
    NVCH = D // VCH

    wq3 = w_qkv.reshape(D, H, DH, 3)
    wq = np.ascontiguousarray(wq3[:, :, :, 0].reshape(D, D))
    wk = np.ascontiguousarray(wq3[:, :, :, 1].reshape(D, D))
    wv = np.ascontiguousarray(wq3[:, :, :, 2].reshape(D, D))
    wp = np.ascontiguousarray(np.asarray(w_proj, np.float32))

    NPBF = mybir.dt.np(mybir.dt.bfloat16)

    def pair_major(w):  # [D, D] -> [P, NPAIR*KT*P], pair-block major
        blocks = [
            _tile_rows(w[:, p * P : (p + 1) * P]) for p in range(NPAIR)
        ]
        return _round_f32r(np.concatenate(blocks, axis=1))

    def chunk_major(w):  # [D, D] -> [P, NVCH*KT*VCH] bf16, chunk major
        blocks = [
            _tile_rows(w[:, c * VCH : (c + 1) * VCH]) for c in range(NVCH)
        ]
        return np.concatenate(blocks, axis=1).astype(NPBF)

    out = {
        "wq": pair_major(wq),
        "wk": pair_major(wk),
        "wv": chunk_major(wv),
        "wp": chunk_major(wp),
    }
    b3 = np.asarray(b_qkv, np.float32).reshape(H, DH, 3)
    bq = np.ascontiguousarray(b3[:, :, 0].reshape(D))
    bk = np.ascontiguousarray(b3[:, :, 1].reshape(D))
    bv = np.ascontiguousarray(b3[:, :, 2].reshape(D))
    out["bq"] = np.ascontiguousarray(bq.reshape(NPAIR, P).T).astype(np.float32)
    out["bk"] = np.ascontiguousarray(bk.reshape(NPAIR, P).T).astype(np.float32)
    out["bv"] = bv.reshape(1, D).astype(NPBF)
    out["bp"] = np.asarray(b_proj, np.float32).reshape(1, D).astype(NPBF)
    return out


def host_prep_x(x_b, TOK, D):
    """One batch element [TOK, D] -> {x: f32r, xv: bf16} tiled [P, KT*TOK]."""
    xT = np.ascontiguousarray(np.asarray(x_b, np.float32).T)  # [D, TOK]
    t = _tile_rows(xT)
    return {
        "x": _round_f32r(t),
        "xv": t.astype(mybir.dt.np(mybir.dt.bfloat16)),
    }


# ---------------------------------------------------------------------------
# entry point
# ---------------------------------------------------------------------------

_BUILT = {}


def _get_nc(TOK, D, H, att_scale):
    key = (TOK, D, H, att_scale)
    if key not in _BUILT:
        nc = bacc.Bacc(
            "TRN2",
            target_bir_lowering=False,
            debug=False,
            dynamic_dma_scratch_size=512,
        )
        build(nc, TOK, D, H, att_scale)
        nc.compile()
        nc.finalize()
        _BUILT[key] = nc
    return _BUILT[key]


def kernel(x, w_qkv, b_qkv, w_proj, b_proj):
    from concourse.bass_utils import run_bass_kernel_spmd

    x = np.asarray(x, np.float32)
    B, TOK, D = x.shape
    H = H_FULL
    shared = host_prep_shared(
        np.asarray(w_qkv, np.float32),
        np.asarray(b_qkv, np.float32),
        np.asarray(w_proj, np.float32),
        np.asarray(b_proj, np.float32),
        D,
        H,
    )
    in_maps = []
    for b in range(B):
        m = dict(shared)
        m.update(host_prep_x(x[b], TOK, D))
        in_maps.append(m)

    nc = _get_nc(TOK, D, H, ATT_SCALE_FULL)
    res = run_bass_kernel_spmd(nc, in_maps, list(range(N_CORES)))
    out = np.stack([res.results[b]["out"] for b in range(B)], axis=0)
    return out.astype(np.float32)


# revision 20
# speedup vs baseline: 2.1905x; 1.0013x over previous
"""Multi-head attention kernel for Trainium2 (Bass/Tile), 8 NeuronCores.

Problem: nn_MultiHeadAttention
  x [8, 1024, 1024] f32, w_qkv [1024, 3072], b_qkv [3072],
  w_proj [1024, 1024], b_proj [1024]  ->  out [8, 1024, 1024]

  qkv = x @ w_qkv + b_qkv ; split (h, d, 3) interleaved on last dim
  score = q k^T per (b, h);  att = softmax(score, -1) / sqrt(1024)
  out = (att @ v) reshaped @ w_proj + b_proj

Sharding: data-parallel over batch. Each of the 8 cores runs the full
MHA for one batch element; no collectives.

v2 design (post-trace): the v1 kernel ran at 680us with the PE
clock-gated to 1.2GHz for 2/3 of the span (HAM re-throttle during
serial per-pair stalls) and 104us of single-partition DVE RECIPROCAL.
This version:
  - keeps the PE warm: attention is an ACT-paced conveyor (scores ->
    exp -> att@V per k-tile, ping-pong PSUM), with the NEXT pair's
    Q/K projections interleaved into the same span so the PE never
    idles long enough to re-throttle.
  - scores run as two concurrent row-tiled (64x128) matmuls: even
    head on PE tile T0 (SBUF rows 0:64), odd head on T8 (rows 64:128).
  - softmax denominators come free from a 65th "32.0" column in the
    V operand (folds the 1/sqrt(D) post-scale); their reciprocals are
    computed 2 rows at a time with reciprocal_approx_fast (~5x faster,
    128 partitions wide) and broadcast across partitions with one
    K=2 indicator matmul.
  - all weight DMA slices are contiguous per partition (per-pair /
    per-chunk major DRAM layout).

Device-side math per core (no on-device transpose anywhere):
  qT = (x wq)^T  [(h,d), tok]   kT likewise
  v_aug = [x wv + bv | 32.0] per head   [tok, h*(d+1)]
  per head pair, per k-tile: S^T[k,q] = kT.T-slice @ qT-slice (T0/T8)
     E = exp(S^T); O'^T[0:64,q] += v_aug.T @ E; O'^T[64,q] = 32*denom
  ao^T = O' * (1/(32*denom))  (recip via DVE approx, bcast via PE)
  out = ao^T.T @ wp + bp
"""

import os

os.environ.setdefault("MYCRO_LOCAL_CACHE", "1")

import numpy as np

import concourse.bass as bass
import concourse.tile as tile
from concourse import bacc, mybir

P = 128
DH = 64  # head dim
F32 = mybir.dt.float32
F32R = mybir.dt.float32r
BF = mybir.dt.bfloat16
F16 = mybir.dt.float16
# matmul-operand dtype for the score path (x, wq/wk, qT/kT): f32r keeps
# 11 mantissa bits, needed because score errors pass through exp().
# Everything else (V, E, attout, proj) runs bf16: same 1 cycle/col PE
# rate but fast weight loads and half the DMA/SBUF footprint.
MM = F32R

# full-problem constants
B_FULL = 8
TOK_FULL = 1024
D_FULL = 1024
H_FULL = 16
ATT_SCALE_FULL = 1.0 / 32.0  # 1/sqrt(1024), applied after softmax
N_CORES = 8


def build(nc, TOK, D, H, att_scale):
    """Emit the one-core MHA program (one batch element).

    DRAM inputs (host pre-laid-out, all slices contiguous/partition):
      x        [P, KT*TOK]        [p][kt][t] = x[t, kt*P + p]
      wq, wk   [P, NPAIR*KT*P]    [p][pair][kt][n]; cols n = pair block
      wv, wp   [P, NVCH*KT*VCH]   [p][c][kt][n];  cols n = chunk block
      bq, bk   [P, NPAIR]         [p][pair] = b[pair*P + p]
      bv, bp   [1, D]
    Output: out [TOK, D] f32
    """
    assert D == H * DH and D % P == 0 and TOK % P == 0 and H % 2 == 0
    KT = D // P        # contraction tiles over the model dim
    MT = TOK // P      # token tiles (also the k-tiles of attention)
    NPAIR = H // 2     # head pairs
    VW = H * (DH + 1)  # v_aug row width: per head [v | aug]
    QCH = min(512, TOK)   # moving-chunk width for scores / att@V / QK
    NQH = TOK // QCH
    VCH = min(512, D)     # column chunk for V / proj weight streaming
    NVCH = D // VCH
    AUG = 1.0 / att_scale  # 32.0: folded post-softmax scale
    EXP = mybir.ActivationFunctionType.Exp
    assert MT % 2 == 0

    x_d = nc.dram_tensor("x", [P, KT * TOK], MM, kind="ExternalInput")
    xv_d = nc.dram_tensor("xv", [P, KT * TOK], BF, kind="ExternalInput")
    wq_d = nc.dram_tensor("wq", [P, NPAIR * KT * P], MM, kind="ExternalInput")
    wk_d = nc.dram_tensor("wk", [P, NPAIR * KT * P], MM, kind="ExternalInput")
    wv_d = nc.dram_tensor("wv", [P, NVCH * KT * VCH], BF, kind="ExternalInput")
    wp_d = nc.dram_tensor("wp", [P, NVCH * KT * VCH], BF, kind="ExternalInput")
    bq_d = nc.dram_tensor("bq", [P, NPAIR], F32, kind="ExternalInput")
    bk_d = nc.dram_tensor("bk", [P, NPAIR], F32, kind="ExternalInput")
    bv_d = nc.dram_tensor("bv", [1, D], BF, kind="ExternalInput")
    bp_d = nc.dram_tensor("bp", [1, D], BF, kind="ExternalInput")
    out_d = nc.dram_tensor("out", [TOK, D], F32, kind="ExternalOutput")

    with tile.TileContext(nc) as tc:
        with (
            tc.tile_pool(name="sing", bufs=1) as sing,
            tc.tile_pool(name="psS", bufs=2, space="PSUM") as psS,
            tc.tile_pool(name="psO", bufs=4, space="PSUM") as psO,
            tc.tile_pool(name="ebuf", bufs=4) as ebuf,
            tc.tile_pool(name="qkp", bufs=2) as qkp,
            tc.tile_pool(name="wqkp", bufs=2) as wqkp,
            tc.tile_pool(name="rbuf", bufs=2) as rbuf,
            tc.tile_pool(name="outp", bufs=2) as outp,
        ):
            from concourse import library_config

            nc.gpsimd.load_library(library_config.attn)

            # ---------------- persistent SBUF ----------------
            # DMA priority order: bf16 V-phase inputs first (smallest
            # path to first matmul), then the f32r score-path inputs.
            xv_sb = sing.tile([P, KT, TOK], BF, tag="xv")
            nc.sync.dma_start(out=xv_sb, in_=xv_d[:, :])

            # memset cannot target f32r/bf16; stage in f32, cast via DVE
            cst_sb = sing.tile([2, P], F32, tag="cst")
            nc.vector.memset(cst_sb, 1.0)
            ones_bf = sing.tile([1, P], BF, tag="ones")
            nc.vector.tensor_copy(out=ones_bf, in_=cst_sb[0:1, :])
            vones_sb = sing.tile([P, MT * H], F32, tag="vones")
            nc.vector.memset(vones_sb, AUG)

            x_sb = sing.tile([P, KT, TOK], MM, tag="x")
            half = KT // 2
            nc.sync.dma_start(
                out=x_sb[:, 0:half, :], in_=x_d[:, 0 : half * TOK]
            )
            nc.sync.dma_start(
                out=x_sb[:, half:KT, :], in_=x_d[:, half * TOK : KT * TOK]
            )
            x3 = x_sb

            bq_sb = sing.tile([P, NPAIR], F32, tag="bq")
            nc.sync.dma_start(out=bq_sb, in_=bq_d[:, :])
            bk_sb = sing.tile([P, NPAIR], F32, tag="bk")
            nc.sync.dma_start(out=bk_sb, in_=bk_d[:, :])
            bv_sb = sing.tile([1, D], BF, tag="bv")
            nc.sync.dma_start(out=bv_sb, in_=bv_d[:, :])
            bp_sb = sing.tile([1, D], BF, tag="bp")
            nc.sync.dma_start(out=bp_sb, in_=bp_d[:, :])

            v_sb = sing.tile([P, MT, VW], BF, tag="v")   # v_aug
            # aug columns (denominator accumulators) = 1/att_scale
            nc.vector.tensor_copy(
                out=v_sb[:, :, :]
                .rearrange("p m (h e) -> p m h e", e=DH + 1)[:, :, :, DH],
                in_=vones_sb[:, :].rearrange("p (m h) -> p m h", h=H),
            )
            ao_sb = sing.tile([P, NPAIR, TOK], BF, tag="ao")  # attout^T

            wv3 = wv_d[:, :].rearrange("p (c kt n) -> p c kt n", c=NVCH, kt=KT)
            wp3 = wp_d[:, :].rearrange("p (c kt n) -> p c kt n", c=NVCH, kt=KT)
            wq3 = wq_d[:, :].rearrange("p (pr kt n) -> p pr kt n", pr=NPAIR, kt=KT)
            wk3 = wk_d[:, :].rearrange("p (pr kt n) -> p pr kt n", pr=NPAIR, kt=KT)

            # ---------------- Q/K projection task ----------------
            # qT/kT for one pair: [P rows = (even|odd head dims), TOK]
            qT = {}
            kT = {}

            def load_wqk(p):
                wq_sb = wqkp.tile([P, KT, P], MM, tag="wq")
                nc.sync.dma_start(
                    out=wq_sb, in_=wq3[:, p, :, :]
                )
                wk_sb = wqkp.tile([P, KT, P], MM, tag="wk")
                nc.sync.dma_start(
                    out=wk_sb, in_=wk3[:, p, :, :]
                )
                return wq_sb, wk_sb

            def emit_qk_task(p, which, w_sb, c0, cw):
                """One accumulation task: (x @ w_pair)^T chunk + bias."""
                if which == "q":
                    if p not in qT:
                        qT[p] = qkp.tile([P, TOK], F16, tag="qT", name=f"qT{p}")
                    dst, b_sb = qT[p], bq_sb
                else:
                    if p not in kT:
                        kT[p] = qkp.tile([P, TOK], F16, tag="kT", name=f"kT{p}")
                    dst, b_sb = kT[p], bk_sb
                ps = psO.tile([P, QCH], F32, tag="O")
                for kt in range(KT):
                    nc.tensor.matmul(
                        ps[:, 0:cw],
                        lhsT=w_sb[:, kt, :],
                        rhs=x3[:, kt, c0 : c0 + cw],
                        start=(kt == 0),
                        stop=(kt == KT - 1),
                    )
                nc.vector.tensor_scalar_add(
                    out=dst[:, c0 : c0 + cw],
                    in0=ps[:, 0:cw],
                    scalar1=b_sb[:, p : p + 1],
                )

            # ---------------- V phase: v_aug = [x wv + bv | AUG] --------
            wv_sb = sing.tile([P, NVCH, KT, VCH], BF, tag="wv")
            nc.sync.dma_start(out=wv_sb, in_=wv3[:, :, :, :])
            wp_sb = sing.tile([P, NVCH, KT, VCH], BF, tag="wp")
            nc.sync.dma_start(out=wp_sb, in_=wp3[:, :, :, :])
            wqk0 = load_wqk(0)
            for c in range(NVCH):
                for mt in range(MT):
                    ps_v = psO.tile([P, QCH], F32, tag="O")
                    for kt in range(KT):
                        nc.tensor.matmul(
                            ps_v[:, 0:VCH],
                            lhsT=xv_sb[:, kt, mt * P : (mt + 1) * P],
                            rhs=wv_sb[:, c, kt, :],
                            start=(kt == 0),
                            stop=False,
                        )
                    nc.tensor.matmul(
                        ps_v[:, 0:VCH],
                        lhsT=ones_bf[0:1, 0:P],
                        rhs=bv_sb[0:1, c * VCH : (c + 1) * VCH],
                        start=False,
                        stop=True,
                    )
                    # scatter heads into v_aug (DH+1 stride)
                    nh = VCH // DH
                    h0 = c * VCH // DH
                    nc.vector.tensor_copy(
                        out=v_sb[:, mt, :]
                        .rearrange("p (h e) -> p h e", e=DH + 1)[
                            :, h0 : h0 + nh, 0:DH
                        ],
                        in_=ps_v[:, 0:VCH].rearrange(
                            "p (h d) -> p h d", d=DH
                        ),
                    )
                # interleave pair-0 Q/K projections into the V phase
                if c == 0:
                    for c0 in range(0, TOK, QCH):
                        emit_qk_task(0, "q", wqk0[0], c0, QCH)
                if c == NVCH - 1 or NVCH == 1:
                    for c0 in range(0, TOK, QCH):
                        emit_qk_task(0, "k", wqk0[1], c0, QCH)

            # ---------------- attention, per head pair ----------------
            # Software-pipelined conveyor: per 2-kb group emit scores,
            # then exp, then the PREVIOUS group's att@V (so the PE queue
            # never sits behind an exp that hasn't finished). The
            # normalize for each (pair, qh) is split: the DVE reciprocal
            # chain is emitted at block end, but the PE broadcast + final
            # muls are deferred into the next block.
            pending_norm = [None]

            def flush_norm():
                if pending_norm[0] is None:
                    return
                p_, q0_, ps_oe_, ps_oo_, rc_ = pending_norm[0]
                pending_norm[0] = None
                # partition-broadcast of both heads' reciprocals (GPSIMD,
                # keeps the PE out of the normalize chain entirely)
                bcs = rbuf.tile([DH, 2 * QCH], BF, tag="bcs")
                nc.gpsimd.partition_broadcast(
                    out_ap=bcs, in_ap=rc_[0:1, :], channels=DH
                )
                nc.vector.tensor_mul(
                    out=ao_sb[0:DH, p_, q0_ : q0_ + QCH],
                    in0=ps_oe_[0:DH, :],
                    in1=bcs[0:DH, 0:QCH],
                )
                nc.vector.tensor_mul(
                    out=ao_sb[DH:P, p_, q0_ : q0_ + QCH],
                    in0=ps_oo_[0:DH, :],
                    in1=bcs[0:DH, QCH : 2 * QCH],
                )

            for p in range(NPAIR):
                wqk_next = load_wqk(p + 1) if p + 1 < NPAIR else None
                for qh in range(NQH):
                    q0 = qh * QCH
                    ps_oe = psO.tile([P, QCH], F32, tag="O", name="ps_oe")
                    ps_oo = psO.tile([P, QCH], F32, tag="O", name="ps_oo")

                    def emit_attv(kbs, e_ts):
                        for kb, et in zip(kbs, e_ts):
                            for hoff, ps_out in ((0, ps_oe), (1, ps_oo)):
                                hh = 2 * p + hoff
                                nc.tensor.matmul(
                                    ps_out[0 : DH + 1, :],
                                    lhsT=v_sb[
                                        :, kb,
                                        hh * (DH + 1) : (hh + 1) * (DH + 1),
                                    ],
                                    rhs=et[:, hoff * QCH : (hoff + 1) * QCH],
                                    start=(kb == 0),
                                    stop=(kb == MT - 1),
                                    skip_group_check=True,
                                )

                    prev = None
                    for g in range(0, MT, 2):
                        kbs = (g, g + 1)
                        e_ts = []
                        for kb in kbs:
                            st = psS.tile([P, 2 * QCH], F32, tag="S")
                            # even head: PE tile T0 (SBUF rows 0:64)
                            # odd head: T8 (rows 64:128) -- concurrent
                            nc.tensor.matmul(
                                st[:, 0:QCH],
                                lhsT=kT[p][0:DH, kb * P : (kb + 1) * P],
                                rhs=qT[p][0:DH, q0 : q0 + QCH],
                                start=True,
                                stop=True,
                            )
                            nc.tensor.matmul(
                                st[:, QCH : 2 * QCH],
                                lhsT=kT[p][DH:P, kb * P : (kb + 1) * P],
                                rhs=qT[p][DH:P, q0 : q0 + QCH],
                                start=True,
                                stop=True,
                            )
                            et = ebuf.tile([P, 2 * QCH], BF, tag="E")
                            nc.scalar.activation(out=et, in_=st, func=EXP)
                            e_ts.append(et)
                        if g == 2:
                            # previous block's deferred PE broadcast+muls
                            flush_norm()
                        if prev is not None:
                            emit_attv(*prev)
                        # next pair's Q (during qh 0) / K (during the last
                        # qh) projection tasks, before the last attV flush
                        if wqk_next is not None and g + 2 >= MT:
                            tasks = []
                            if qh == 0:
                                tasks.append(("q", wqk_next[0]))
                            if qh == NQH - 1:
                                tasks.append(("k", wqk_next[1]))
                            for which, w_sb in tasks:
                                for c0 in range(0, TOK, QCH):
                                    emit_qk_task(p + 1, which, w_sb, c0, QCH)
                        prev = (kbs, e_ts)
                    emit_attv(*prev)

                    # ---- normalize DVE chain: 1/(32*denom) ----
                    rq = rbuf.tile([1, 2 * QCH], F32, tag="rq")
                    nc.vector.tensor_copy(
                        out=rq[0:1, 0:QCH], in_=ps_oe[DH : DH + 1, :]
                    )
                    nc.vector.tensor_copy(
                        out=rq[0:1, QCH : 2 * QCH],
                        in_=ps_oo[DH : DH + 1, :],
                    )
                    rr = rbuf.tile([1, 2 * QCH], F32, tag="rr")
                    nc.vector.reciprocal_approx_fast(out=rr, in_=rq)
                    rc = rbuf.tile([1, 2 * QCH], BF, tag="rc")
                    nc.vector.tensor_copy(out=rc, in_=rr)
                    flush_norm()  # no-op unless MT==2 left it pending
                    pending_norm[0] = (p, q0, ps_oe, ps_oo, rc)
            flush_norm()

            # ---------------- projection: out = ao^T.T wp + bp -------
            # kt order is pair-completion order, so the scheduler can
            # run the first KT-1 accumulation steps of each output tile
            # during the last pair's attention.
            for c in range(NVCH):
                for mt in range(MT):
                    ps_p = psO.tile([P, QCH], F32, tag="O")
                    for kt in range(KT):
                        nc.tensor.matmul(
                            ps_p[:, 0:VCH],
                            lhsT=ao_sb[:, kt, mt * P : (mt + 1) * P],
                            rhs=wp_sb[:, c, kt, :],
                            start=(kt == 0),
                            stop=False,
                        )
                    nc.tensor.matmul(
                        ps_p[:, 0:VCH],
                        lhsT=ones_bf[0:1, 0:P],
                        rhs=bp_sb[0:1, c * VCH : (c + 1) * VCH],
                        start=False,
                        stop=True,
                    )
                    o_sb = outp.tile([P, VCH], F32, tag="o")
                    nc.vector.tensor_copy(
                        out=o_sb[:, 0:VCH], in_=ps_p[:, 0:VCH]
                    )
                    nc.sync.dma_start(
                        out=out_d[
                            mt * P : (mt + 1) * P,
                            c * VCH : (c + 1) * VCH,
                        ],
                        in_=o_sb[:, 0:VCH],
                    )

    return nc


# ---------------------------------------------------------------------------
# host-side layout prep
# ---------------------------------------------------------------------------

def _round_f32r(x):
    """RNE to f32r's 11-explicit-mantissa-bit grid (matches HW rounding)."""
    u = np.ascontiguousarray(x, np.float32).view(np.uint32)
    u = ((u + np.uint32(1 << 11)) >> 12) << 12
    return u.view(np.float32)


def _tile_rows(w):
    """[D, N] -> [P, (D//P) * N] with [p][kt][n] layout."""
    Dd, N = w.shape
    KT = Dd // P
    return np.ascontiguousarray(
        w.reshape(KT, P, N).transpose(1, 0, 2).reshape(P, KT * N)
    )


def host_prep_shared(w_qkv, b_qkv, w_proj, b_proj, D, H):
    """Split/retile the weights once for all cores."""
    NPAIR = H // 2
    VCH = min(512, D)
    NVCH = D // VCH

    wq3 = w_qkv.reshape(D, H, DH, 3)
    wq = np.ascontiguousarray(wq3[:, :, :, 0].reshape(D, D))
    wk = np.ascontiguousarray(wq3[:, :, :, 1].reshape(D, D))
    wv = np.ascontiguousarray(wq3[:, :, :, 2].reshape(D, D))
    wp = np.ascontiguousarray(np.asarray(w_proj, np.float32))

    NPBF = mybir.dt.np(mybir.dt.bfloat16)

    def pair_major(w):  # [D, D] -> [P, NPAIR*KT*P], pair-block major
        blocks = [
            _tile_rows(w[:, p * P : (p + 1) * P]) for p in range(NPAIR)
        ]
        return _round_f32r(np.concatenate(blocks, axis=1))

    def chunk_major(w):  # [D, D] -> [P, NVCH*KT*VCH] bf16, chunk major
        blocks = [
            _tile_rows(w[:, c * VCH : (c + 1) * VCH]) for c in range(NVCH)
        ]
        return np.concatenate(blocks, axis=1).astype(NPBF)

    out = {
        "wq": pair_major(wq),
        "wk": pair_major(wk),
        "wv": chunk_major(wv),
        "wp": chunk_major(wp),
    }
    b3 = np.asarray(b_qkv, np.float32).reshape(H, DH, 3)
    bq = np.ascontiguousarray(b3[:, :, 0].reshape(D))
    bk = np.ascontiguousarray(b3[:, :, 1].reshape(D))
    bv = np.ascontiguousarray(b3[:, :, 2].reshape(D))
    out["bq"] = np.ascontiguousarray(bq.reshape(NPAIR, P).T).astype(np.float32)
    out["bk"] = np.ascontiguousarray(bk.reshape(NPAIR, P).T).astype(np.float32)
    out["bv"] = bv.reshape(1, D).astype(NPBF)
    out["bp"] = np.asarray(b_proj, np.float32).reshape(1, D).astype(NPBF)
    return out


def host_prep_x(x_b, TOK, D):
    """One batch element [TOK, D] -> {x: f32r, xv: bf16} tiled [P, KT*TOK]."""
    xT = np.ascontiguousarray(np.asarray(x_b, np.float32).T)  # [D, TOK]
    t = _tile_rows(xT)
    return {
        "x": _round_f32r(t),
        "xv": t.astype(mybir.dt.np(mybir.dt.bfloat16)),
    }


# ---------------------------------------------------------------------------
# entry point
# ---------------------------------------------------------------------------

_BUILT = {}


def _get_nc(TOK, D, H, att_scale):
    key = (TOK, D, H, att_scale)
    if key not in _BUILT:
        nc = bacc.Bacc(
            "TRN2",
            target_bir_lowering=False,
            debug=False,
            dynamic_dma_scratch_size=512,
        )
        build(nc, TOK, D, H, att_scale)
        nc.compile()
        nc.finalize()
        _BUILT[key] = nc
    return _BUILT[key]


def kernel(x, w_qkv, b_qkv, w_proj, b_proj):
    from concourse.bass_utils import run_bass_kernel_spmd

    x = np.asarray(x, np.float32)
    B, TOK, D = x.shape
    H = H_FULL
    shared = host_prep_shared(
        np.asarray(w_qkv, np.float32),
        np.asarray(b_qkv, np.float32),
        np.asarray(w_proj, np.float32),
        np.asarray(b_proj, np.float32),
        D,
        H,
    )
    in_maps = []
    for b in range(B):
        m = dict(shared)
        m.update(host_prep_x(x[b], TOK, D))
        in_maps.append(m)

    nc = _get_nc(TOK, D, H, ATT_SCALE_FULL)
    res = run_bass_kernel_spmd(nc, in_maps, list(range(N_CORES)))
    out = np.stack([res.results[b]["out"] for b in range(B)], axis=0)
    return out.astype(np.float32)


# revision 21
# speedup vs baseline: 2.2555x; 1.0297x over previous
"""Multi-head attention kernel for Trainium2 (Bass/Tile), 8 NeuronCores.

Problem: nn_MultiHeadAttention
  x [8, 1024, 1024] f32, w_qkv [1024, 3072], b_qkv [3072],
  w_proj [1024, 1024], b_proj [1024]  ->  out [8, 1024, 1024]

  qkv = x @ w_qkv + b_qkv ; split (h, d, 3) interleaved on last dim
  score = q k^T per (b, h);  att = softmax(score, -1) / sqrt(1024)
  out = (att @ v) reshaped @ w_proj + b_proj

Sharding: data-parallel over batch. Each of the 8 cores runs the full
MHA for one batch element; no collectives.

v2 design (post-trace): the v1 kernel ran at 680us with the PE
clock-gated to 1.2GHz for 2/3 of the span (HAM re-throttle during
serial per-pair stalls) and 104us of single-partition DVE RECIPROCAL.
This version:
  - keeps the PE warm: attention is an ACT-paced conveyor (scores ->
    exp -> att@V per k-tile, ping-pong PSUM), with the NEXT pair's
    Q/K projections interleaved into the same span so the PE never
    idles long enough to re-throttle.
  - scores run as two concurrent row-tiled (64x128) matmuls: even
    head on PE tile T0 (SBUF rows 0:64), odd head on T8 (rows 64:128).
  - softmax denominators come free from a 65th "32.0" column in the
    V operand (folds the 1/sqrt(D) post-scale); their reciprocals are
    computed 2 rows at a time with reciprocal_approx_fast (~5x faster,
    128 partitions wide) and broadcast across partitions with one
    K=2 indicator matmul.
  - all weight DMA slices are contiguous per partition (per-pair /
    per-chunk major DRAM layout).

Device-side math per core (no on-device transpose anywhere):
  qT = (x wq)^T  [(h,d), tok]   kT likewise
  v_aug = [x wv + bv | 32.0] per head   [tok, h*(d+1)]
  per head pair, per k-tile: S^T[k,q] = kT.T-slice @ qT-slice (T0/T8)
     E = exp(S^T); O'^T[0:64,q] += v_aug.T @ E; O'^T[64,q] = 32*denom
  ao^T = O' * (1/(32*denom))  (recip via DVE approx, bcast via PE)
  out = ao^T.T @ wp + bp
"""

import os

os.environ.setdefault("MYCRO_LOCAL_CACHE", "1")

import numpy as np

import concourse.bass as bass
import concourse.tile as tile
from concourse import bacc, mybir

P = 128
DH = 64  # head dim
F32 = mybir.dt.float32
F32R = mybir.dt.float32r
BF = mybir.dt.bfloat16
F16 = mybir.dt.float16
# matmul-operand dtype for the score path (x, wq/wk, qT/kT): f32r keeps
# 11 mantissa bits, needed because score errors pass through exp().
# Everything else (V, E, attout, proj) runs bf16: same 1 cycle/col PE
# rate but fast weight loads and half the DMA/SBUF footprint.
MM = F32R

# full-problem constants
B_FULL = 8
TOK_FULL = 1024
D_FULL = 1024
H_FULL = 16
ATT_SCALE_FULL = 1.0 / 32.0  # 1/sqrt(1024), applied after softmax
N_CORES = 8


def build(nc, TOK, D, H, att_scale):
    """Emit the one-core MHA program (one batch element).

    DRAM inputs (host pre-laid-out, all slices contiguous/partition):
      x        [P, KT*TOK]        [p][kt][t] = x[t, kt*P + p]
      wq, wk   [P, NPAIR*KT*P]    [p][pair][kt][n]; cols n = pair block
      wv, wp   [P, NVCH*KT*VCH]   [p][c][kt][n];  cols n = chunk block
      bq, bk   [P, NPAIR]         [p][pair] = b[pair*P + p]
      bv, bp   [1, D]
    Output: out [TOK, D] f32
    """
    assert D == H * DH and D % P == 0 and TOK % P == 0 and H % 2 == 0
    KT = D // P        # contraction tiles over the model dim
    MT = TOK // P      # token tiles (also the k-tiles of attention)
    NPAIR = H // 2     # head pairs
    VW = H * (DH + 1)  # v_aug row width: per head [v | aug]
    QCH = min(512, TOK)   # moving-chunk width for scores / att@V / QK
    NQH = TOK // QCH
    VCH = min(512, D)     # column chunk for V / proj weight streaming
    NVCH = D // VCH
    AUG = 1.0 / att_scale  # 32.0: folded post-softmax scale
    EXP = mybir.ActivationFunctionType.Exp
    assert MT % 2 == 0

    x_d = nc.dram_tensor("x", [P, KT * TOK], MM, kind="ExternalInput")
    xv_d = nc.dram_tensor("xv", [P, KT * TOK], BF, kind="ExternalInput")
    wq_d = nc.dram_tensor("wq", [P, NPAIR * KT * P], MM, kind="ExternalInput")
    wk_d = nc.dram_tensor("wk", [P, NPAIR * KT * P], MM, kind="ExternalInput")
    wv_d = nc.dram_tensor("wv", [P, NVCH * KT * VCH], BF, kind="ExternalInput")
    wp_d = nc.dram_tensor("wp", [P, NVCH * KT * VCH], BF, kind="ExternalInput")
    bq_d = nc.dram_tensor("bq", [P, NPAIR], F32, kind="ExternalInput")
    bk_d = nc.dram_tensor("bk", [P, NPAIR], F32, kind="ExternalInput")
    bv_d = nc.dram_tensor("bv", [1, D], BF, kind="ExternalInput")
    bp_d = nc.dram_tensor("bp", [1, D], BF, kind="ExternalInput")
    out_d = nc.dram_tensor("out", [TOK, D], F32, kind="ExternalOutput")

    with tile.TileContext(nc) as tc:
        with (
            tc.tile_pool(name="sing", bufs=1) as sing,
            tc.tile_pool(name="psS", bufs=2, space="PSUM") as psS,
            tc.tile_pool(name="psO", bufs=4, space="PSUM") as psO,
            tc.tile_pool(name="ebuf", bufs=4) as ebuf,
            tc.tile_pool(name="qkp", bufs=2) as qkp,
            tc.tile_pool(name="wqkp", bufs=2) as wqkp,
            tc.tile_pool(name="rbuf", bufs=2) as rbuf,
            tc.tile_pool(name="outp", bufs=2) as outp,
        ):
            from concourse import library_config

            nc.gpsimd.load_library(library_config.attn)

            # ---------------- persistent SBUF ----------------
            # DMA priority order: bf16 V-phase inputs first (smallest
            # path to first matmul), then the f32r score-path inputs.
            xv_sb = sing.tile([P, KT, TOK], BF, tag="xv")
            nc.sync.dma_start(out=xv_sb, in_=xv_d[:, :])
            wv3 = wv_d[:, :].rearrange("p (c kt n) -> p c kt n", c=NVCH, kt=KT)
            wv_sb = sing.tile([P, NVCH, KT, VCH], BF, tag="wv")
            for c in range(NVCH):
                nc.sync.dma_start(out=wv_sb[:, c, :, :], in_=wv3[:, c, :, :])

            # memset cannot target f32r/bf16; stage in f32, cast via DVE
            cst_sb = sing.tile([2, P], F32, tag="cst")
            nc.vector.memset(cst_sb, 1.0)
            ones_bf = sing.tile([1, P], BF, tag="ones")
            nc.vector.tensor_copy(out=ones_bf, in_=cst_sb[0:1, :])
            vones_sb = sing.tile([P, MT * H], F32, tag="vones")
            nc.vector.memset(vones_sb, AUG)

            x_sb = sing.tile([P, KT, TOK], MM, tag="x")
            half = KT // 2
            nc.sync.dma_start(
                out=x_sb[:, 0:half, :], in_=x_d[:, 0 : half * TOK]
            )
            nc.sync.dma_start(
                out=x_sb[:, half:KT, :], in_=x_d[:, half * TOK : KT * TOK]
            )
            x3 = x_sb

            bq_sb = sing.tile([P, NPAIR], F32, tag="bq")
            nc.sync.dma_start(out=bq_sb, in_=bq_d[:, :])
            bk_sb = sing.tile([P, NPAIR], F32, tag="bk")
            nc.sync.dma_start(out=bk_sb, in_=bk_d[:, :])
            bv_sb = sing.tile([1, D], BF, tag="bv")
            nc.sync.dma_start(out=bv_sb, in_=bv_d[:, :])
            bp_sb = sing.tile([1, D], BF, tag="bp")
            nc.sync.dma_start(out=bp_sb, in_=bp_d[:, :])

            v_sb = sing.tile([P, MT, VW], BF, tag="v")   # v_aug
            # aug columns (denominator accumulators) = 1/att_scale
            nc.vector.tensor_copy(
                out=v_sb[:, :, :]
                .rearrange("p m (h e) -> p m h e", e=DH + 1)[:, :, :, DH],
                in_=vones_sb[:, :].rearrange("p (m h) -> p m h", h=H),
            )
            ao_sb = sing.tile([P, NPAIR, TOK], BF, tag="ao")  # attout^T

            wp3 = wp_d[:, :].rearrange("p (c kt n) -> p c kt n", c=NVCH, kt=KT)
            wq3 = wq_d[:, :].rearrange("p (pr kt n) -> p pr kt n", pr=NPAIR, kt=KT)
            wk3 = wk_d[:, :].rearrange("p (pr kt n) -> p pr kt n", pr=NPAIR, kt=KT)

            # ---------------- Q/K projection task ----------------
            # qT/kT for one pair: [P rows = (even|odd head dims), TOK]
            qT = {}
            kT = {}

            def load_wqk(p):
                wq_sb = wqkp.tile([P, KT, P], MM, tag="wq")
                nc.sync.dma_start(
                    out=wq_sb, in_=wq3[:, p, :, :]
                )
                wk_sb = wqkp.tile([P, KT, P], MM, tag="wk")
                nc.sync.dma_start(
                    out=wk_sb, in_=wk3[:, p, :, :]
                )
                return wq_sb, wk_sb

            def emit_qk_task(p, which, w_sb, c0, cw):
                """One accumulation task: (x @ w_pair)^T chunk + bias."""
                if which == "q":
                    if p not in qT:
                        qT[p] = qkp.tile([P, TOK], F16, tag="qT", name=f"qT{p}")
                    dst, b_sb = qT[p], bq_sb
                else:
                    if p not in kT:
                        kT[p] = qkp.tile([P, TOK], F16, tag="kT", name=f"kT{p}")
                    dst, b_sb = kT[p], bk_sb
                ps = psS.tile([P, QCH], F32, tag="S", name="ps_qk")
                for kt in range(KT):
                    nc.tensor.matmul(
                        ps[:, 0:cw],
                        lhsT=w_sb[:, kt, :],
                        rhs=x3[:, kt, c0 : c0 + cw],
                        start=(kt == 0),
                        stop=(kt == KT - 1),
                    )
                nc.vector.tensor_scalar_add(
                    out=dst[:, c0 : c0 + cw],
                    in0=ps[:, 0:cw],
                    scalar1=b_sb[:, p : p + 1],
                )

            # ---------------- V phase: v_aug = [x wv + bv | AUG] --------
            wp_sb = sing.tile([P, NVCH, KT, VCH], BF, tag="wp")
            wqk0 = load_wqk(0)
            for c in range(NVCH):
                for mt in range(MT):
                    ps_v = psO.tile([P, QCH], F32, tag="O")
                    for kt in range(KT):
                        nc.tensor.matmul(
                            ps_v[:, 0:VCH],
                            lhsT=xv_sb[:, kt, mt * P : (mt + 1) * P],
                            rhs=wv_sb[:, c, kt, :],
                            start=(kt == 0),
                            stop=False,
                        )
                    nc.tensor.matmul(
                        ps_v[:, 0:VCH],
                        lhsT=ones_bf[0:1, 0:P],
                        rhs=bv_sb[0:1, c * VCH : (c + 1) * VCH],
                        start=False,
                        stop=True,
                    )
                    # scatter heads into v_aug (DH+1 stride)
                    nh = VCH // DH
                    h0 = c * VCH // DH
                    nc.vector.tensor_copy(
                        out=v_sb[:, mt, :]
                        .rearrange("p (h e) -> p h e", e=DH + 1)[
                            :, h0 : h0 + nh, 0:DH
                        ],
                        in_=ps_v[:, 0:VCH].rearrange(
                            "p (h d) -> p h d", d=DH
                        ),
                    )
                # interleave pair-0 Q/K projections into the V phase
                if c == 0:
                    for c0 in range(0, TOK, QCH):
                        emit_qk_task(0, "q", wqk0[0], c0, QCH)
                if c == NVCH - 1 or NVCH == 1:
                    for c0 in range(0, TOK, QCH):
                        emit_qk_task(0, "k", wqk0[1], c0, QCH)

            nc.sync.dma_start(out=wp_sb, in_=wp3[:, :, :, :])

            # ---------------- attention, per head pair ----------------
            # Software-pipelined conveyor: per 2-kb group emit scores,
            # then exp, then the PREVIOUS group's att@V (so the PE queue
            # never sits behind an exp that hasn't finished). The
            # normalize for each (pair, qh) is split: the DVE reciprocal
            # chain is emitted at block end, but the PE broadcast + final
            # muls are deferred into the next block.
            pending_norm = [None]

            def flush_norm():
                if pending_norm[0] is None:
                    return
                p_, q0_, ps_oe_, ps_oo_, rc_ = pending_norm[0]
                pending_norm[0] = None
                # partition-broadcast of both heads' reciprocals (GPSIMD,
                # keeps the PE out of the normalize chain entirely)
                bcs = rbuf.tile([DH, 2 * QCH], BF, tag="bcs")
                nc.gpsimd.partition_broadcast(
                    out_ap=bcs, in_ap=rc_[0:1, :], channels=DH
                )
                nc.vector.tensor_mul(
                    out=ao_sb[0:DH, p_, q0_ : q0_ + QCH],
                    in0=ps_oe_[0:DH, :],
                    in1=bcs[0:DH, 0:QCH],
                )
                nc.vector.tensor_mul(
                    out=ao_sb[DH:P, p_, q0_ : q0_ + QCH],
                    in0=ps_oo_[0:DH, :],
                    in1=bcs[0:DH, QCH : 2 * QCH],
                )

            for p in range(NPAIR):
                wqk_next = load_wqk(p + 1) if p + 1 < NPAIR else None
                for qh in range(NQH):
                    q0 = qh * QCH
                    ps_oe = psO.tile([P, QCH], F32, tag="O", name="ps_oe")
                    ps_oo = psO.tile([P, QCH], F32, tag="O", name="ps_oo")

                    def emit_attv(kbs, e_ts):
                        for kb, et in zip(kbs, e_ts):
                            for hoff, ps_out in ((0, ps_oe), (1, ps_oo)):
                                hh = 2 * p + hoff
                                nc.tensor.matmul(
                                    ps_out[0 : DH + 1, :],
                                    lhsT=v_sb[
                                        :, kb,
                                        hh * (DH + 1) : (hh + 1) * (DH + 1),
                                    ],
                                    rhs=et[:, hoff * QCH : (hoff + 1) * QCH],
                                    start=(kb == 0),
                                    stop=(kb == MT - 1),
                                    skip_group_check=True,
                                )

                    prev = None
                    for g in range(0, MT, 2):
                        kbs = (g, g + 1)
                        e_ts = []
                        for kb in kbs:
                            st = psS.tile([P, 2 * QCH], F32, tag="S")
                            # even head: PE tile T0 (SBUF rows 0:64)
                            # odd head: T8 (rows 64:128) -- concurrent
                            nc.tensor.matmul(
                                st[:, 0:QCH],
                                lhsT=kT[p][0:DH, kb * P : (kb + 1) * P],
                                rhs=qT[p][0:DH, q0 : q0 + QCH],
                                start=True,
                                stop=True,
                            )
                            nc.tensor.matmul(
                                st[:, QCH : 2 * QCH],
                                lhsT=kT[p][DH:P, kb * P : (kb + 1) * P],
                                rhs=qT[p][DH:P, q0 : q0 + QCH],
                                start=True,
                                stop=True,
                            )
                            et = ebuf.tile([P, 2 * QCH], BF, tag="E")
                            nc.scalar.activation(out=et, in_=st, func=EXP)
                            e_ts.append(et)
                        if g == 0:
                            # previous block's deferred broadcast+muls
                            flush_norm()
                        if prev is not None:
                            emit_attv(*prev)
                        # next pair's Q (during qh 0) / K (during the last
                        # qh) projection tasks, before the last attV flush
                        if wqk_next is not None and g + 2 >= MT:
                            tasks = []
                            if qh == 0:
                                tasks.append(("q", wqk_next[0]))
                            if qh == NQH - 1:
                                tasks.append(("k", wqk_next[1]))
                            for which, w_sb in tasks:
                                for c0 in range(0, TOK, QCH):
                                    emit_qk_task(p + 1, which, w_sb, c0, QCH)
                        prev = (kbs, e_ts)
                    emit_attv(*prev)

                    # ---- normalize DVE chain: 1/(32*denom) ----
                    rq = rbuf.tile([1, 2 * QCH], F32, tag="rq")
                    nc.vector.tensor_copy(
                        out=rq[0:1, 0:QCH], in_=ps_oe[DH : DH + 1, :]
                    )
                    nc.vector.tensor_copy(
                        out=rq[0:1, QCH : 2 * QCH],
                        in_=ps_oo[DH : DH + 1, :],
                    )
                    rr = rbuf.tile([1, 2 * QCH], F32, tag="rr")
                    nc.vector.reciprocal_approx_fast(out=rr, in_=rq)
                    rc = rbuf.tile([1, 2 * QCH], BF, tag="rc")
                    nc.vector.tensor_copy(out=rc, in_=rr)
                    flush_norm()  # no-op unless MT==2 left it pending
                    pending_norm[0] = (p, q0, ps_oe, ps_oo, rc)
            flush_norm()

            # ---------------- projection: out = ao^T.T wp + bp -------
            # kt order is pair-completion order, so the scheduler can
            # run the first KT-1 accumulation steps of each output tile
            # during the last pair's attention.
            for c in range(NVCH):
                for mt in range(MT):
                    ps_p = psO.tile([P, QCH], F32, tag="O")
                    for kt in range(KT):
                        nc.tensor.matmul(
                            ps_p[:, 0:VCH],
                            lhsT=ao_sb[:, kt, mt * P : (mt + 1) * P],
                            rhs=wp_sb[:, c, kt, :],
                            start=(kt == 0),
                            stop=False,
                        )
                    nc.tensor.matmul(
                        ps_p[:, 0:VCH],
                        lhsT=ones_bf[0:1, 0:P],
                        rhs=bp_sb[0:1, c * VCH : (c + 1) * VCH],
                        start=False,
                        stop=True,
                    )
                    o_sb = outp.tile([P, VCH], F32, tag="o")
                    nc.vector.tensor_copy(
                        out=o_sb[:, 0:VCH], in_=ps_p[:, 0:VCH]
                    )
                    nc.sync.dma_start(
                        out=out_d[
                            mt * P : (mt + 1) * P,
                            c * VCH : (c + 1) * VCH,
                        ],
                        in_=o_sb[:, 0:VCH],
                    )

    return nc


# ---------------------------------------------------------------------------
# host-side layout prep
# ---------------------------------------------------------------------------

def _round_f32r(x):
    """RNE to f32r's 11-explicit-mantissa-bit grid (matches HW rounding)."""
    u = np.ascontiguousarray(x, np.float32).view(np.uint32)
    u = ((u + np.uint32(1 << 11)) >> 12) << 12
    return u.view(np.float32)


def _tile_rows(w):
    """[D, N] -> [P, (D//P) * N] with [p][kt][n] layout."""
    Dd, N = w.shape
    KT = Dd // P
    return np.ascontiguousarray(
        w.reshape(KT, P, N).transpose(1, 0, 2).reshape(P, KT * N)
    )


def host_prep_shared(w_qkv, b_qkv, w_proj, b_proj, D, H):
    """Split/retile the weights once for all cores."""
    NPAIR = H // 2
    VCH = min(512, D)
    NVCH = D // VCH

    wq3 = w_qkv.reshape(D, H, DH, 3)
    wq = np.ascontiguousarray(wq3[:, :, :, 0].reshape(D, D))
    wk = np.ascontiguousarray(wq3[:, :, :, 1].reshape(D, D))
    wv = np.ascontiguousarray(wq3[:, :, :, 2].reshape(D, D))
    wp = np.ascontiguousarray(np.asarray(w_proj, np.float32))

    NPBF = mybir.dt.np(mybir.dt.bfloat16)

    def pair_major(w):  # [D, D] -> [P, NPAIR*KT*P], pair-block major
        blocks = [
            _tile_rows(w[:, p * P : (p + 1) * P]) for p in range(NPAIR)
        ]
        return _round_f32r(np.concatenate(blocks, axis=1))

    def chunk_major(w):  # [D, D] -> [P, NVCH*KT*VCH] bf16, chunk major
        blocks = [
            _tile_rows(w[:, c * VCH : (c + 1) * VCH]) for c in range(NVCH)
        ]
        return np.concatenate(blocks, axis=1).astype(NPBF)

    out = {
        "wq": pair_major(wq),
        "wk": pair_major(wk),
        "wv": chunk_major(wv),
        "wp": chunk_major(wp),
    }
    b3 = np.asarray(b_qkv, np.float32).reshape(H, DH, 3)
    bq = np.ascontiguousarray(b3[:, :, 0].reshape(D))
    bk = np.ascontiguousarray(b3[:, :, 1].reshape(D))
    bv = np.ascontiguousarray(b3[:, :, 2].reshape(D))
    out["bq"] = np.ascontiguousarray(bq.reshape(NPAIR, P).T).astype(np.float32)
    out["bk"] = np.ascontiguousarray(bk.reshape(NPAIR, P).T).astype(np.float32)
    out["bv"] = bv.reshape(1, D).astype(NPBF)
    out["bp"] = np.asarray(b_proj, np.float32).reshape(1, D).astype(NPBF)
    return out


def host_prep_x(x_b, TOK, D):
    """One batch element [TOK, D] -> {x: f32r, xv: bf16} tiled [P, KT*TOK]."""
    xT = np.ascontiguousarray(np.asarray(x_b, np.float32).T)  # [D, TOK]
    t = _tile_rows(xT)
    return {
        "x": _round_f32r(t),
        "xv": t.astype(mybir.dt.np(mybir.dt.bfloat16)),
    }


# ---------------------------------------------------------------------------
# entry point
# ---------------------------------------------------------------------------

_BUILT = {}


def _get_nc(TOK, D, H, att_scale):
    key = (TOK, D, H, att_scale)
    if key not in _BUILT:
        nc = bacc.Bacc(
            "TRN2",
            target_bir_lowering=False,
            debug=False,
            dynamic_dma_scratch_size=512,
        )
        build(nc, TOK, D, H, att_scale)
        nc.compile()
        nc.finalize()
        _BUILT[key] = nc
    return _BUILT[key]


def kernel(x, w_qkv, b_qkv, w_proj, b_proj):
    from concourse.bass_utils import run_bass_kernel_spmd

    x = np.asarray(x, np.float32)
    B, TOK, D = x.shape
    H = H_FULL
    shared = host_prep_shared(
        np.asarray(w_qkv, np.float32),
        np.asarray(b_qkv, np.float32),
        np.asarray(w_proj, np.float32),
        np.asarray(b_proj, np.float32),
        D,
        H,
    )
    in_maps = []
    for b in range(B):
        m = dict(shared)
        m.update(host_prep_x(x[b], TOK, D))
        in_maps.append(m)

    nc = _get_nc(TOK, D, H, ATT_SCALE_FULL)
    res = run_bass_kernel_spmd(nc, in_maps, list(range(N_CORES)))
    out = np.stack([res.results[b]["out"] for b in range(B)], axis=0)
    return out.astype(np.float32)


# revision 23
# speedup vs baseline: 2.2940x; 1.0171x over previous
"""Multi-head attention kernel for Trainium2 (Bass/Tile), 8 NeuronCores.

Problem: nn_MultiHeadAttention
  x [8, 1024, 1024] f32, w_qkv [1024, 3072], b_qkv [3072],
  w_proj [1024, 1024], b_proj [1024]  ->  out [8, 1024, 1024]

  qkv = x @ w_qkv + b_qkv ; split (h, d, 3) interleaved on last dim
  score = q k^T per (b, h);  att = softmax(score, -1) / sqrt(1024)
  out = (att @ v) reshaped @ w_proj + b_proj

Sharding: data-parallel over batch. Each of the 8 cores runs the full
MHA for one batch element; no collectives.

v2 design (post-trace): the v1 kernel ran at 680us with the PE
clock-gated to 1.2GHz for 2/3 of the span (HAM re-throttle during
serial per-pair stalls) and 104us of single-partition DVE RECIPROCAL.
This version:
  - keeps the PE warm: attention is an ACT-paced conveyor (scores ->
    exp -> att@V per k-tile, ping-pong PSUM), with the NEXT pair's
    Q/K projections interleaved into the same span so the PE never
    idles long enough to re-throttle.
  - scores run as two concurrent row-tiled (64x128) matmuls: even
    head on PE tile T0 (SBUF rows 0:64), odd head on T8 (rows 64:128).
  - softmax denominators come free from a 65th "32.0" column in the
    V operand (folds the 1/sqrt(D) post-scale); their reciprocals are
    computed 2 rows at a time with reciprocal_approx_fast (~5x faster,
    128 partitions wide) and broadcast across partitions with one
    K=2 indicator matmul.
  - all weight DMA slices are contiguous per partition (per-pair /
    per-chunk major DRAM layout).

Device-side math per core (no on-device transpose anywhere):
  qT = (x wq)^T  [(h,d), tok]   kT likewise
  v_aug = [x wv + bv | 32.0] per head   [tok, h*(d+1)]
  per head pair, per k-tile: S^T[k,q] = kT.T-slice @ qT-slice (T0/T8)
     E = exp(S^T); O'^T[0:64,q] += v_aug.T @ E; O'^T[64,q] = 32*denom
  ao^T = O' * (1/(32*denom))  (recip via DVE approx, bcast via PE)
  out = ao^T.T @ wp + bp
"""

import os

os.environ.setdefault("MYCRO_LOCAL_CACHE", "1")

import numpy as np

import concourse.bass as bass
import concourse.tile as tile
from concourse import bacc, mybir

P = 128
DH = 64  # head dim
F32 = mybir.dt.float32
F32R = mybir.dt.float32r
BF = mybir.dt.bfloat16
F16 = mybir.dt.float16
# matmul-operand dtype for the score path (x, wq/wk, qT/kT): f32r keeps
# 11 mantissa bits, needed because score errors pass through exp().
# Everything else (V, E, attout, proj) runs bf16: same 1 cycle/col PE
# rate but fast weight loads and half the DMA/SBUF footprint.
MM = F32R

# full-problem constants
B_FULL = 8
TOK_FULL = 1024
D_FULL = 1024
H_FULL = 16
ATT_SCALE_FULL = 1.0 / 32.0  # 1/sqrt(1024), applied after softmax
N_CORES = 8


def build(nc, TOK, D, H, att_scale):
    """Emit the one-core MHA program (one batch element).

    DRAM inputs (host pre-laid-out, all slices contiguous/partition):
      x        [P, KT*TOK]        [p][kt][t] = x[t, kt*P + p]
      wq, wk   [P, NPAIR*KT*P]    [p][pair][kt][n]; cols n = pair block
      wv, wp   [P, NVCH*KT*VCH]   [p][c][kt][n];  cols n = chunk block
      bq, bk   [P, NPAIR]         [p][pair] = b[pair*P + p]
      bv, bp   [1, D]
    Output: out [TOK, D] f32
    """
    assert D == H * DH and D % P == 0 and TOK % P == 0 and H % 2 == 0
    KT = D // P        # contraction tiles over the model dim
    MT = TOK // P      # token tiles (also the k-tiles of attention)
    NPAIR = H // 2     # head pairs
    VW = H * (DH + 1)  # v_aug row width: per head [v | aug]
    QCH = min(512, TOK)   # moving-chunk width for scores / att@V / QK
    NQH = TOK // QCH
    VCH = min(512, D)     # column chunk for V / proj weight streaming
    NVCH = D // VCH
    AUG = 1.0 / att_scale  # 32.0: folded post-softmax scale
    EXP = mybir.ActivationFunctionType.Exp
    assert MT % 2 == 0

    x_d = nc.dram_tensor("x", [P, KT * TOK], MM, kind="ExternalInput")
    xv_d = nc.dram_tensor("xv", [P, KT * TOK], BF, kind="ExternalInput")
    wq_d = nc.dram_tensor("wq", [P, NPAIR * KT * P], MM, kind="ExternalInput")
    wk_d = nc.dram_tensor("wk", [P, NPAIR * KT * P], MM, kind="ExternalInput")
    wv_d = nc.dram_tensor("wv", [P, NVCH * KT * VCH], BF, kind="ExternalInput")
    wp_d = nc.dram_tensor("wp", [P, NVCH * KT * VCH], BF, kind="ExternalInput")
    bq_d = nc.dram_tensor("bq", [P, NPAIR], F32, kind="ExternalInput")
    bk_d = nc.dram_tensor("bk", [P, NPAIR], F32, kind="ExternalInput")
    bv_d = nc.dram_tensor("bv", [1, D], BF, kind="ExternalInput")
    bp_d = nc.dram_tensor("bp", [1, D], BF, kind="ExternalInput")
    out_d = nc.dram_tensor("out", [TOK, D], F32, kind="ExternalOutput")

    with tile.TileContext(nc) as tc:
        with (
            tc.tile_pool(name="sing", bufs=1) as sing,
            tc.tile_pool(name="psS", bufs=2, space="PSUM") as psS,
            tc.tile_pool(name="psO", bufs=4, space="PSUM") as psO,
            tc.tile_pool(name="ebuf", bufs=4) as ebuf,
            tc.tile_pool(name="qkp", bufs=2) as qkp,
            tc.tile_pool(name="wqkp", bufs=2) as wqkp,
            tc.tile_pool(name="rbuf", bufs=2) as rbuf,
            tc.tile_pool(name="outp", bufs=2) as outp,
        ):
            from concourse import library_config

            nc.gpsimd.load_library(library_config.attn)

            # ---------------- persistent SBUF ----------------
            # DMA priority order: bf16 V-phase inputs first (smallest
            # path to first matmul), then the f32r score-path inputs.
            xv_sb = sing.tile([P, KT, TOK], BF, tag="xv")
            nc.sync.dma_start(out=xv_sb, in_=xv_d[:, :])
            wv3 = wv_d[:, :].rearrange("p (c kt n) -> p c kt n", c=NVCH, kt=KT)
            wv_sb = sing.tile([P, NVCH, KT, VCH], BF, tag="wv")
            for c in range(NVCH):
                nc.sync.dma_start(out=wv_sb[:, c, :, :], in_=wv3[:, c, :, :])

            # memset cannot target f32r/bf16; stage in f32, cast via DVE
            cst_sb = sing.tile([2, P], F32, tag="cst")
            nc.vector.memset(cst_sb, 1.0)
            ones_bf = sing.tile([1, P], BF, tag="ones")
            nc.vector.tensor_copy(out=ones_bf, in_=cst_sb[0:1, :])
            vones_sb = sing.tile([P, MT * H], F32, tag="vones")
            nc.vector.memset(vones_sb, AUG)

            x_sb = sing.tile([P, KT, TOK], MM, tag="x")
            half = KT // 2
            nc.sync.dma_start(
                out=x_sb[:, 0:half, :], in_=x_d[:, 0 : half * TOK]
            )
            nc.sync.dma_start(
                out=x_sb[:, half:KT, :], in_=x_d[:, half * TOK : KT * TOK]
            )
            x3 = x_sb

            bq_sb = sing.tile([P, NPAIR], F32, tag="bq")
            nc.sync.dma_start(out=bq_sb, in_=bq_d[:, :])
            bk_sb = sing.tile([P, NPAIR], F32, tag="bk")
            nc.sync.dma_start(out=bk_sb, in_=bk_d[:, :])
            bv_sb = sing.tile([1, D], BF, tag="bv")
            nc.sync.dma_start(out=bv_sb, in_=bv_d[:, :])
            bp_sb = sing.tile([1, D], BF, tag="bp")
            nc.sync.dma_start(out=bp_sb, in_=bp_d[:, :])

            v_sb = sing.tile([P, MT, VW], BF, tag="v")   # v_aug
            # aug columns (denominator accumulators) = 1/att_scale
            nc.vector.tensor_copy(
                out=v_sb[:, :, :]
                .rearrange("p m (h e) -> p m h e", e=DH + 1)[:, :, :, DH],
                in_=vones_sb[:, :].rearrange("p (m h) -> p m h", h=H),
            )
            ao_sb = sing.tile([P, NPAIR, TOK], BF, tag="ao")  # attout^T

            wp3 = wp_d[:, :].rearrange("p (c kt n) -> p c kt n", c=NVCH, kt=KT)
            wq3 = wq_d[:, :].rearrange("p (pr kt n) -> p pr kt n", pr=NPAIR, kt=KT)
            wk3 = wk_d[:, :].rearrange("p (pr kt n) -> p pr kt n", pr=NPAIR, kt=KT)

            # ---------------- Q/K projection task ----------------
            # qT/kT for one pair: [P rows = (even|odd head dims), TOK]
            qT = {}
            kT = {}

            def load_wqk(p):
                wq_sb = wqkp.tile([P, KT, P], MM, tag="wq")
                nc.sync.dma_start(
                    out=wq_sb, in_=wq3[:, p, :, :]
                )
                wk_sb = wqkp.tile([P, KT, P], MM, tag="wk")
                nc.sync.dma_start(
                    out=wk_sb, in_=wk3[:, p, :, :]
                )
                return wq_sb, wk_sb

            def emit_qk_task(p, which, w_sb, c0, cw):
                """One accumulation task: (x @ w_pair)^T chunk + bias."""
                if which == "q":
                    if p not in qT:
                        qT[p] = qkp.tile([P, TOK], F16, tag="qT", name=f"qT{p}")
                    dst, b_sb = qT[p], bq_sb
                else:
                    if p not in kT:
                        kT[p] = qkp.tile([P, TOK], F16, tag="kT", name=f"kT{p}")
                    dst, b_sb = kT[p], bk_sb
                ps = psS.tile([P, QCH], F32, tag="S", name="ps_qk")
                for kt in range(KT):
                    nc.tensor.matmul(
                        ps[:, 0:cw],
                        lhsT=w_sb[:, kt, :],
                        rhs=x3[:, kt, c0 : c0 + cw],
                        start=(kt == 0),
                        stop=(kt == KT - 1),
                    )
                nc.vector.tensor_scalar_add(
                    out=dst[:, c0 : c0 + cw],
                    in0=ps[:, 0:cw],
                    scalar1=b_sb[:, p : p + 1],
                )

            # ---------------- V phase: v_aug = [x wv + bv | AUG] --------
            wp_sb = sing.tile([P, NVCH, KT, VCH], BF, tag="wp")
            wqk0 = load_wqk(0)
            for c in range(NVCH):
                for mt in range(MT):
                    ps_v = psO.tile([P, QCH], F32, tag="O")
                    for kt in range(KT):
                        nc.tensor.matmul(
                            ps_v[:, 0:VCH],
                            lhsT=xv_sb[:, kt, mt * P : (mt + 1) * P],
                            rhs=wv_sb[:, c, kt, :],
                            start=(kt == 0),
                            stop=False,
                        )
                    nc.tensor.matmul(
                        ps_v[:, 0:VCH],
                        lhsT=ones_bf[0:1, 0:P],
                        rhs=bv_sb[0:1, c * VCH : (c + 1) * VCH],
                        start=False,
                        stop=True,
                    )
                    # scatter heads into v_aug (DH+1 stride)
                    nh = VCH // DH
                    h0 = c * VCH // DH
                    nc.vector.tensor_copy(
                        out=v_sb[:, mt, :]
                        .rearrange("p (h e) -> p h e", e=DH + 1)[
                            :, h0 : h0 + nh, 0:DH
                        ],
                        in_=ps_v[:, 0:VCH].rearrange(
                            "p (h d) -> p h d", d=DH
                        ),
                    )
                # interleave pair-0 Q/K projections into the late V
                # phase (the f32r x they consume lands behind xv/wv)
                if c == NVCH - 1:
                    for c0 in range(0, TOK, QCH):
                        emit_qk_task(0, "q", wqk0[0], c0, QCH)
                    for c0 in range(0, TOK, QCH):
                        emit_qk_task(0, "k", wqk0[1], c0, QCH)

            nc.sync.dma_start(out=wp_sb, in_=wp3[:, :, :, :])

            # ---------------- attention, per head pair ----------------
            # Software-pipelined conveyor: per 2-kb group emit scores,
            # then exp, then the PREVIOUS group's att@V (so the PE queue
            # never sits behind an exp that hasn't finished). The
            # normalize for each (pair, qh) is split: the DVE reciprocal
            # chain is emitted at block end, but the PE broadcast + final
            # muls are deferred into the next block.
            pending_norm = []

            def flush_norm():
                if not pending_norm:
                    return
                p_, q0_, ps_oe_, ps_oo_, rc_ = pending_norm.pop(0)
                # partition-broadcast of both heads' reciprocals (GPSIMD,
                # keeps the PE out of the normalize chain entirely)
                bcs = rbuf.tile([DH, 2 * QCH], BF, tag="bcs")
                nc.gpsimd.partition_broadcast(
                    out_ap=bcs, in_ap=rc_[0:1, :], channels=DH
                )
                nc.vector.tensor_mul(
                    out=ao_sb[0:DH, p_, q0_ : q0_ + QCH],
                    in0=ps_oe_[0:DH, :],
                    in1=bcs[0:DH, 0:QCH],
                )
                nc.vector.tensor_mul(
                    out=ao_sb[DH:P, p_, q0_ : q0_ + QCH],
                    in0=ps_oo_[0:DH, :],
                    in1=bcs[0:DH, QCH : 2 * QCH],
                )

            for p in range(NPAIR):
                wqk_next = load_wqk(p + 1) if p + 1 < NPAIR else None
                # one continuous conveyor over all (qh, kb) units of the
                # pair -- no drain between the q-halves
                units = [(qh, kb) for qh in range(NQH) for kb in range(MT)]
                NU = len(units)
                ps_o = {}

                def get_o(qh):
                    if qh not in ps_o:
                        oe = psO.tile([P, QCH], F32, tag="O", name="ps_oe")
                        oo = psO.tile([P, QCH], F32, tag="O", name="ps_oo")
                        ps_o[qh] = (oe, oo)
                    return ps_o[qh]

                def emit_attv(us, e_ts):
                    for (qh, kb), et in zip(us, e_ts):
                        for hoff, ps_out in ((0, get_o(qh)[0]),
                                             (1, get_o(qh)[1])):
                            hh = 2 * p + hoff
                            nc.tensor.matmul(
                                ps_out[0 : DH + 1, :],
                                lhsT=v_sb[
                                    :, kb,
                                    hh * (DH + 1) : (hh + 1) * (DH + 1),
                                ],
                                rhs=et[:, hoff * QCH : (hoff + 1) * QCH],
                                start=(kb == 0),
                                stop=(kb == MT - 1),
                                skip_group_check=True,
                            )

                def emit_norm_chain(qh):
                    ps_oe, ps_oo = get_o(qh)
                    rq = rbuf.tile([1, 2 * QCH], F32, tag="rq")
                    nc.vector.tensor_copy(
                        out=rq[0:1, 0:QCH], in_=ps_oe[DH : DH + 1, :]
                    )
                    nc.vector.tensor_copy(
                        out=rq[0:1, QCH : 2 * QCH],
                        in_=ps_oo[DH : DH + 1, :],
                    )
                    rr = rbuf.tile([1, 2 * QCH], F32, tag="rr")
                    nc.vector.reciprocal_approx_fast(out=rr, in_=rq)
                    rc = rbuf.tile([1, 2 * QCH], BF, tag="rc")
                    nc.vector.tensor_copy(out=rc, in_=rr)
                    pending_norm.append((p, qh * QCH, ps_oe, ps_oo, rc))

                gq = (NU // 2) & ~1
                gk = max(0, NU - 4)
                prev = None
                for g in range(0, NU, 2):
                    us = units[g : g + 2]
                    e_ts = []
                    for qh, kb in us:
                        q0 = qh * QCH
                        st = psS.tile([P, 2 * QCH], F32, tag="S")
                        # even head: PE tile T0 (SBUF rows 0:64)
                        # odd head: T8 (rows 64:128) -- concurrent
                        nc.tensor.matmul(
                            st[:, 0:QCH],
                            lhsT=kT[p][0:DH, kb * P : (kb + 1) * P],
                            rhs=qT[p][0:DH, q0 : q0 + QCH],
                            start=True,
                            stop=True,
                        )
                        nc.tensor.matmul(
                            st[:, QCH : 2 * QCH],
                            lhsT=kT[p][DH:P, kb * P : (kb + 1) * P],
                            rhs=qT[p][DH:P, q0 : q0 + QCH],
                            start=True,
                            stop=True,
                        )
                        et = ebuf.tile([P, 2 * QCH], BF, tag="E")
                        nc.scalar.activation(out=et, in_=st, func=EXP)
                        e_ts.append(et)
                    if g == 0:
                        # previous pair's deferred qh1 broadcast+muls
                        flush_norm()
                    if g == MT + 4:
                        # this pair's qh0 broadcast+muls
                        flush_norm()
                    if prev is not None:
                        emit_attv(*prev)
                        if prev[0][-1][1] == MT - 1:
                            emit_norm_chain(prev[0][-1][0])
                    if wqk_next is not None:
                        if g == gq:
                            for c0 in range(0, TOK, QCH):
                                emit_qk_task(p + 1, "q", wqk_next[0], c0, QCH)
                        if g == gk:
                            for c0 in range(0, TOK, QCH):
                                emit_qk_task(p + 1, "k", wqk_next[1], c0, QCH)
                    prev = (us, e_ts)
                emit_attv(*prev)
                if prev[0][-1][1] == MT - 1:
                    emit_norm_chain(prev[0][-1][0])
            while pending_norm:
                flush_norm()

            # ---------------- projection: out = ao^T.T wp + bp -------
            # kt order is pair-completion order, so the scheduler can
            # run the first KT-1 accumulation steps of each output tile
            # during the last pair's attention.
            for c in range(NVCH):
                for mt in range(MT):
                    ps_p = psO.tile([P, QCH], F32, tag="O")
                    for kt in range(KT):
                        nc.tensor.matmul(
                            ps_p[:, 0:VCH],
                            lhsT=ao_sb[:, kt, mt * P : (mt + 1) * P],
                            rhs=wp_sb[:, c, kt, :],
                            start=(kt == 0),
                            stop=False,
                        )
                    nc.tensor.matmul(
                        ps_p[:, 0:VCH],
                        lhsT=ones_bf[0:1, 0:P],
                        rhs=bp_sb[0:1, c * VCH : (c + 1) * VCH],
                        start=False,
                        stop=True,
                    )
                    o_sb = outp.tile([P, VCH], F32, tag="o")
                    nc.vector.tensor_copy(
                        out=o_sb[:, 0:VCH], in_=ps_p[:, 0:VCH]
                    )
                    nc.sync.dma_start(
                        out=out_d[
                            mt * P : (mt + 1) * P,
                            c * VCH : (c + 1) * VCH,
                        ],
                        in_=o_sb[:, 0:VCH],
                    )

    return nc


# ---------------------------------------------------------------------------
# host-side layout prep
# ---------------------------------------------------------------------------

def _round_f32r(x):
    """RNE to f32r's 11-explicit-mantissa-bit grid (matches HW rounding)."""
    u = np.ascontiguousarray(x, np.float32).view(np.uint32)
    u = ((u + np.uint32(1 << 11)) >> 12) << 12
    return u.view(np.float32)


def _tile_rows(w):
    """[D, N] -> [P, (D//P) * N] with [p][kt][n] layout."""
    Dd, N = w.shape
    KT = Dd // P
    return np.ascontiguousarray(
        w.reshape(KT, P, N).transpose(1, 0, 2).reshape(P, KT * N)
    )


def host_prep_shared(w_qkv, b_qkv, w_proj, b_proj, D, H):
    """Split/retile the weights once for all cores."""
    NPAIR = H // 2
    VCH = min(512, D)
    NVCH = D // VCH

    wq3 = w_qkv.reshape(D, H, DH, 3)
    wq = np.ascontiguousarray(wq3[:, :, :, 0].reshape(D, D))
    wk = np.ascontiguousarray(wq3[:, :, :, 1].reshape(D, D))
    wv = np.ascontiguousarray(wq3[:, :, :, 2].reshape(D, D))
    wp = np.ascontiguousarray(np.asarray(w_proj, np.float32))

    NPBF = mybir.dt.np(mybir.dt.bfloat16)

    def pair_major(w):  # [D, D] -> [P, NPAIR*KT*P], pair-block major
        blocks = [
            _tile_rows(w[:, p * P : (p + 1) * P]) for p in range(NPAIR)
        ]
        return _round_f32r(np.concatenate(blocks, axis=1))

    def chunk_major(w):  # [D, D] -> [P, NVCH*KT*VCH] bf16, chunk major
        blocks = [
            _tile_rows(w[:, c * VCH : (c + 1) * VCH]) for c in range(NVCH)
        ]
        return np.concatenate(blocks, axis=1).astype(NPBF)

    out = {
        "wq": pair_major(wq),
        "wk": pair_major(wk),
        "wv": chunk_major(wv),
        "wp": chunk_major(wp),
    }
    b3 = np.asarray(b_qkv, np.float32).reshape(H, DH, 3)
    bq = np.ascontiguousarray(b3[:, :, 0].reshape(D))
    bk = np.ascontiguousarray(b3[:, :, 1].reshape(D))
    bv = np.ascontiguousarray(b3[:, :, 2].reshape(D))
    out["bq"] = np.ascontiguousarray(bq.reshape(NPAIR, P).T).astype(np.float32)
    out["bk"] = np.ascontiguousarray(bk.reshape(NPAIR, P).T).astype(np.float32)
    out["bv"] = bv.reshape(1, D).astype(NPBF)
    out["bp"] = np.asarray(b_proj, np.float32).reshape(1, D).astype(NPBF)
    return out


def host_prep_x(x_b, TOK, D):
    """One batch element [TOK, D] -> {x: f32r, xv: bf16} tiled [P, KT*TOK]."""
    xT = np.ascontiguousarray(np.asarray(x_b, np.float32).T)  # [D, TOK]
    t = _tile_rows(xT)
    return {
        "x": _round_f32r(t),
        "xv": t.astype(mybir.dt.np(mybir.dt.bfloat16)),
    }


# ---------------------------------------------------------------------------
# entry point
# ---------------------------------------------------------------------------

_BUILT = {}


def _get_nc(TOK, D, H, att_scale):
    key = (TOK, D, H, att_scale)
    if key not in _BUILT:
        nc = bacc.Bacc(
            "TRN2",
            target_bir_lowering=False,
            debug=False,
            dynamic_dma_scratch_size=512,
        )
        build(nc, TOK, D, H, att_scale)
        nc.compile()
        nc.finalize()
        _BUILT[key] = nc
    return _BUILT[key]


def kernel(x, w_qkv, b_qkv, w_proj, b_proj):
    from concourse.bass_utils import run_bass_kernel_spmd

    x = np.asarray(x, np.float32)
    B, TOK, D = x.shape
    H = H_FULL
    shared = host_prep_shared(
        np.asarray(w_qkv, np.float32),
        np.asarray(b_qkv, np.float32),
        np.asarray(w_proj, np.float32),
        np.asarray(b_proj, np.float32),
        D,
        H,
    )
    in_maps = []
    for b in range(B):
        m = dict(shared)
        m.update(host_prep_x(x[b], TOK, D))
        in_maps.append(m)

    nc = _get_nc(TOK, D, H, ATT_SCALE_FULL)
    res = run_bass_kernel_spmd(nc, in_maps, list(range(N_CORES)))
    out = np.stack([res.results[b]["out"] for b in range(B)], axis=0)
    return out.astype(np.float32)


# revision 25
# speedup vs baseline: 2.4325x; 1.0604x over previous
"""Multi-head attention kernel for Trainium2 (Bass/Tile), 8 NeuronCores.

Problem: nn_MultiHeadAttention
  x [8, 1024, 1024] f32, w_qkv [1024, 3072], b_qkv [3072],
  w_proj [1024, 1024], b_proj [1024]  ->  out [8, 1024, 1024]

  qkv = x @ w_qkv + b_qkv ; split (h, d, 3) interleaved on last dim
  score = q k^T per (b, h);  att = softmax(score, -1) / sqrt(1024)
  out = (att @ v) reshaped @ w_proj + b_proj

Sharding: data-parallel over batch. Each of the 8 cores runs the full
MHA for one batch element; no collectives.

v2 design (post-trace): the v1 kernel ran at 680us with the PE
clock-gated to 1.2GHz for 2/3 of the span (HAM re-throttle during
serial per-pair stalls) and 104us of single-partition DVE RECIPROCAL.
This version:
  - keeps the PE warm: attention is an ACT-paced conveyor (scores ->
    exp -> att@V per k-tile, ping-pong PSUM), with the NEXT pair's
    Q/K projections interleaved into the same span so the PE never
    idles long enough to re-throttle.
  - scores run as two concurrent row-tiled (64x128) matmuls: even
    head on PE tile T0 (SBUF rows 0:64), odd head on T8 (rows 64:128).
  - softmax denominators come free from a 65th "32.0" column in the
    V operand (folds the 1/sqrt(D) post-scale); their reciprocals are
    computed 2 rows at a time with reciprocal_approx_fast (~5x faster,
    128 partitions wide) and broadcast across partitions with one
    K=2 indicator matmul.
  - all weight DMA slices are contiguous per partition (per-pair /
    per-chunk major DRAM layout).

Device-side math per core (no on-device transpose anywhere):
  qT = (x wq)^T  [(h,d), tok]   kT likewise
  v_aug = [x wv + bv | 32.0] per head   [tok, h*(d+1)]
  per head pair, per k-tile: S^T[k,q] = kT.T-slice @ qT-slice (T0/T8)
     E = exp(S^T); O'^T[0:64,q] += v_aug.T @ E; O'^T[64,q] = 32*denom
  ao^T = O' * (1/(32*denom))  (recip via DVE approx, bcast via PE)
  out = ao^T.T @ wp + bp
"""

import os

os.environ.setdefault("MYCRO_LOCAL_CACHE", "1")

import numpy as np

import concourse.bass as bass
import concourse.tile as tile
from concourse import bacc, mybir

P = 128
DH = 64  # head dim
F32 = mybir.dt.float32
F32R = mybir.dt.float32r
BF = mybir.dt.bfloat16
F16 = mybir.dt.float16
# matmul-operand dtype for the score path (x, wq/wk, qT/kT): f32r keeps
# 11 mantissa bits, needed because score errors pass through exp().
# Everything else (V, E, attout, proj) runs bf16: same 1 cycle/col PE
# rate but fast weight loads and half the DMA/SBUF footprint.
MM = F32R

# full-problem constants
B_FULL = 8
TOK_FULL = 1024
D_FULL = 1024
H_FULL = 16
ATT_SCALE_FULL = 1.0 / 32.0  # 1/sqrt(1024), applied after softmax
N_CORES = 8


def build(nc, TOK, D, H, att_scale):
    """Emit the one-core MHA program (one batch element).

    DRAM inputs (host pre-laid-out, all slices contiguous/partition):
      x        [P, KT*TOK]        [p][kt][t] = x[t, kt*P + p]
      wq, wk   [P, NPAIR*KT*P]    [p][pair][kt][n]; cols n = pair block
      wv, wp   [P, NVCH*KT*VCH]   [p][c][kt][n];  cols n = chunk block
      bq, bk   [P, NPAIR]         [p][pair] = b[pair*P + p]
      bv, bp   [1, D]
    Output: out [TOK, D] f32
    """
    assert D == H * DH and D % P == 0 and TOK % P == 0 and H % 2 == 0
    KT = D // P        # contraction tiles over the model dim
    MT = TOK // P      # token tiles (also the k-tiles of attention)
    NPAIR = H // 2     # head pairs
    VW = H * (DH + 1)  # v_aug row width: per head [v | aug]
    QCH = min(512, TOK)   # moving-chunk width for scores / att@V / QK
    NQH = TOK // QCH
    VCH = min(512, D)     # column chunk for V / proj weight streaming
    NVCH = D // VCH
    AUG = 1.0 / att_scale  # 32.0: folded post-softmax scale
    EXP = mybir.ActivationFunctionType.Exp
    assert MT % 2 == 0

    x_d = nc.dram_tensor("x", [P, KT * TOK], F16, kind="ExternalInput")
    xv_d = nc.dram_tensor("xv", [P, KT * TOK], BF, kind="ExternalInput")
    wq_d = nc.dram_tensor("wq", [P, NPAIR * KT * P], F16, kind="ExternalInput")
    wk_d = nc.dram_tensor("wk", [P, NPAIR * KT * P], F16, kind="ExternalInput")
    wv_d = nc.dram_tensor("wv", [P, NVCH * KT * VCH], BF, kind="ExternalInput")
    wp_d = nc.dram_tensor("wp", [P, NVCH * KT * VCH], BF, kind="ExternalInput")
    bq_d = nc.dram_tensor("bq", [P, NPAIR], F32, kind="ExternalInput")
    bk_d = nc.dram_tensor("bk", [P, NPAIR], F32, kind="ExternalInput")
    bv_d = nc.dram_tensor("bv", [1, D], BF, kind="ExternalInput")
    bp_d = nc.dram_tensor("bp", [1, D], BF, kind="ExternalInput")
    out_d = nc.dram_tensor("out", [TOK, D], F32, kind="ExternalOutput")

    with tile.TileContext(nc) as tc:
        with (
            tc.tile_pool(name="sing", bufs=1) as sing,
            tc.tile_pool(name="psS", bufs=2, space="PSUM") as psS,
            tc.tile_pool(name="psO", bufs=4, space="PSUM") as psO,
            tc.tile_pool(name="ebuf", bufs=4) as ebuf,
            tc.tile_pool(name="qkp", bufs=2) as qkp,
            tc.tile_pool(name="wqkp", bufs=2) as wqkp,
            tc.tile_pool(name="rbuf", bufs=2) as rbuf,
            tc.tile_pool(name="outp", bufs=2) as outp,
        ):
            from concourse import library_config

            nc.gpsimd.load_library(library_config.attn)

            # ---------------- persistent SBUF ----------------
            # DMA priority order: bf16 V-phase inputs first (smallest
            # path to first matmul), then the f32r score-path inputs.
            xv_sb = sing.tile([P, KT, TOK], BF, tag="xv")
            nc.sync.dma_start(out=xv_sb, in_=xv_d[:, :])
            wv3 = wv_d[:, :].rearrange("p (c kt n) -> p c kt n", c=NVCH, kt=KT)
            wv_sb = sing.tile([P, NVCH, KT, VCH], BF, tag="wv")
            for c in range(NVCH):
                nc.sync.dma_start(out=wv_sb[:, c, :, :], in_=wv3[:, c, :, :])

            # memset cannot target f32r/bf16; stage in f32, cast via DVE
            cst_sb = sing.tile([2, P], F32, tag="cst")
            nc.vector.memset(cst_sb, 1.0)
            ones_bf = sing.tile([1, P], BF, tag="ones")
            nc.vector.tensor_copy(out=ones_bf, in_=cst_sb[0:1, :])
            vones_sb = sing.tile([P, MT * H], F32, tag="vones")
            nc.vector.memset(vones_sb, AUG)

            x_sb = sing.tile([P, KT, TOK], F16, tag="x")
            half = KT // 2
            nc.sync.dma_start(
                out=x_sb[:, 0:half, :], in_=x_d[:, 0 : half * TOK]
            )
            nc.sync.dma_start(
                out=x_sb[:, half:KT, :], in_=x_d[:, half * TOK : KT * TOK]
            )
            x3 = x_sb

            bq_sb = sing.tile([P, NPAIR], F32, tag="bq")
            nc.sync.dma_start(out=bq_sb, in_=bq_d[:, :])
            bk_sb = sing.tile([P, NPAIR], F32, tag="bk")
            nc.sync.dma_start(out=bk_sb, in_=bk_d[:, :])
            bv_sb = sing.tile([1, D], BF, tag="bv")
            nc.sync.dma_start(out=bv_sb, in_=bv_d[:, :])
            bp_sb = sing.tile([1, D], BF, tag="bp")
            nc.sync.dma_start(out=bp_sb, in_=bp_d[:, :])

            v_sb = sing.tile([P, MT, VW], BF, tag="v")   # v_aug
            # aug columns (denominator accumulators) = 1/att_scale
            nc.vector.tensor_copy(
                out=v_sb[:, :, :]
                .rearrange("p m (h e) -> p m h e", e=DH + 1)[:, :, :, DH],
                in_=vones_sb[:, :].rearrange("p (m h) -> p m h", h=H),
            )
            ao_sb = sing.tile([P, NPAIR, TOK], BF, tag="ao")  # attout^T

            wp3 = wp_d[:, :].rearrange("p (c kt n) -> p c kt n", c=NVCH, kt=KT)
            wq3 = wq_d[:, :].rearrange("p (pr kt n) -> p pr kt n", pr=NPAIR, kt=KT)
            wk3 = wk_d[:, :].rearrange("p (pr kt n) -> p pr kt n", pr=NPAIR, kt=KT)

            # ---------------- Q/K projection task ----------------
            # qT/kT for one pair: [P rows = (even|odd head dims), TOK]
            qT = {}
            kT = {}

            def load_wqk(p):
                wq_sb = wqkp.tile([P, KT, P], F16, tag="wq")
                nc.sync.dma_start(
                    out=wq_sb, in_=wq3[:, p, :, :]
                )
                wk_sb = wqkp.tile([P, KT, P], F16, tag="wk")
                nc.sync.dma_start(
                    out=wk_sb, in_=wk3[:, p, :, :]
                )
                return wq_sb, wk_sb

            def emit_qk_task(p, which, w_sb):
                """One accumulation task: (x @ w_pair)^T full width + bias."""
                if which == "q":
                    if p not in qT:
                        qT[p] = qkp.tile([P, TOK], F16, tag="qT", name=f"qT{p}")
                    dst, b_sb = qT[p], bq_sb
                else:
                    if p not in kT:
                        kT[p] = qkp.tile([P, TOK], F16, tag="kT", name=f"kT{p}")
                    dst, b_sb = kT[p], bk_sb
                for t0 in range(0, TOK, QCH):
                    tw = min(QCH, TOK - t0)
                    ps = psS.tile([P, 2 * QCH], F32, tag="S", name="ps_qk")
                    for kt in range(KT):
                        nc.tensor.matmul(
                            ps[:, 0:tw],
                            lhsT=w_sb[:, kt, :],
                            rhs=x3[:, kt, t0 : t0 + tw],
                            start=(kt == 0),
                            stop=(kt == KT - 1),
                        )
                    nc.vector.tensor_scalar_add(
                        out=dst[:, t0 : t0 + tw],
                        in0=ps[:, 0:tw],
                        scalar1=b_sb[:, p : p + 1],
                    )

            # ---------------- V phase: v_aug = [x wv + bv | AUG] --------
            wp_sb = sing.tile([P, NVCH, KT, VCH], BF, tag="wp")
            wqk0 = load_wqk(0)
            for c in range(NVCH):
                for mt in range(MT):
                    ps_v = psO.tile([P, QCH], F32, tag="O")
                    for kt in range(KT):
                        nc.tensor.matmul(
                            ps_v[:, 0:VCH],
                            lhsT=xv_sb[:, kt, mt * P : (mt + 1) * P],
                            rhs=wv_sb[:, c, kt, :],
                            start=(kt == 0),
                            stop=False,
                        )
                    nc.tensor.matmul(
                        ps_v[:, 0:VCH],
                        lhsT=ones_bf[0:1, 0:P],
                        rhs=bv_sb[0:1, c * VCH : (c + 1) * VCH],
                        start=False,
                        stop=True,
                    )
                    # scatter heads into v_aug (DH+1 stride)
                    nh = VCH // DH
                    h0 = c * VCH // DH
                    nc.vector.tensor_copy(
                        out=v_sb[:, mt, :]
                        .rearrange("p (h e) -> p h e", e=DH + 1)[
                            :, h0 : h0 + nh, 0:DH
                        ],
                        in_=ps_v[:, 0:VCH].rearrange(
                            "p (h d) -> p h d", d=DH
                        ),
                    )
                # interleave pair-0 Q/K projections into the late V
                # phase (the f32r x they consume lands behind xv/wv)
                if c == NVCH - 1:
                    emit_qk_task(0, "q", wqk0[0])
                    emit_qk_task(0, "k", wqk0[1])

            nc.sync.dma_start(out=wp_sb, in_=wp3[:, :, :, :])

            # ---------------- attention, per head pair ----------------
            # Software-pipelined conveyor: per 2-kb group emit scores,
            # then exp, then the PREVIOUS group's att@V (so the PE queue
            # never sits behind an exp that hasn't finished). The
            # normalize for each (pair, qh) is split: the DVE reciprocal
            # chain is emitted at block end, but the PE broadcast + final
            # muls are deferred into the next block.
            pending_norm = []

            def flush_norm():
                if not pending_norm:
                    return
                p_, q0_, ps_oe_, ps_oo_, rc_ = pending_norm.pop(0)
                # partition-broadcast of both heads' reciprocals (GPSIMD,
                # keeps the PE out of the normalize chain entirely)
                bcs = rbuf.tile([DH, 2 * QCH], BF, tag="bcs")
                nc.gpsimd.partition_broadcast(
                    out_ap=bcs, in_ap=rc_[0:1, :], channels=DH
                )
                nc.vector.tensor_mul(
                    out=ao_sb[0:DH, p_, q0_ : q0_ + QCH],
                    in0=ps_oe_[0:DH, :],
                    in1=bcs[0:DH, 0:QCH],
                )
                nc.vector.tensor_mul(
                    out=ao_sb[DH:P, p_, q0_ : q0_ + QCH],
                    in0=ps_oo_[0:DH, :],
                    in1=bcs[0:DH, QCH : 2 * QCH],
                )

            for p in range(NPAIR):
                wqk_next = load_wqk(p + 1) if p + 1 < NPAIR else None
                # one continuous conveyor over all (qh, kb) units of the
                # pair -- no drain between the q-halves
                units = [(qh, kb) for qh in range(NQH) for kb in range(MT)]
                NU = len(units)
                ps_o = {}

                def get_o(qh):
                    if qh not in ps_o:
                        oe = psO.tile([P, QCH], F32, tag="O", name="ps_oe")
                        oo = psO.tile([P, QCH], F32, tag="O", name="ps_oo")
                        ps_o[qh] = (oe, oo)
                    return ps_o[qh]

                def emit_attv(us, e_ts):
                    for (qh, kb), et in zip(us, e_ts):
                        for hoff, ps_out in ((0, get_o(qh)[0]),
                                             (1, get_o(qh)[1])):
                            hh = 2 * p + hoff
                            nc.tensor.matmul(
                                ps_out[0 : DH + 1, :],
                                lhsT=v_sb[
                                    :, kb,
                                    hh * (DH + 1) : (hh + 1) * (DH + 1),
                                ],
                                rhs=et[:, hoff * QCH : (hoff + 1) * QCH],
                                start=(kb == 0),
                                stop=(kb == MT - 1),
                                skip_group_check=True,
                            )

                def emit_norm_chain(qh):
                    ps_oe, ps_oo = get_o(qh)
                    rq = rbuf.tile([1, 2 * QCH], F32, tag="rq")
                    nc.vector.tensor_copy(
                        out=rq[0:1, 0:QCH], in_=ps_oe[DH : DH + 1, :]
                    )
                    nc.vector.tensor_copy(
                        out=rq[0:1, QCH : 2 * QCH],
                        in_=ps_oo[DH : DH + 1, :],
                    )
                    rr = rbuf.tile([1, 2 * QCH], F32, tag="rr")
                    nc.vector.reciprocal_approx_fast(out=rr, in_=rq)
                    rc = rbuf.tile([1, 2 * QCH], BF, tag="rc")
                    nc.vector.tensor_copy(out=rc, in_=rr)
                    pending_norm.append((p, qh * QCH, ps_oe, ps_oo, rc))

                gq = (NU // 2) & ~1
                gk = max(0, NU - 4)
                prev = None
                for g in range(0, NU, 2):
                    us = units[g : g + 2]
                    e_ts = []
                    for qh, kb in us:
                        q0 = qh * QCH
                        st = psS.tile([P, 2 * QCH], F32, tag="S")
                        # even head: PE tile T0 (SBUF rows 0:64)
                        # odd head: T8 (rows 64:128) -- concurrent
                        nc.tensor.matmul(
                            st[:, 0:QCH],
                            lhsT=kT[p][0:DH, kb * P : (kb + 1) * P],
                            rhs=qT[p][0:DH, q0 : q0 + QCH],
                            start=True,
                            stop=True,
                        )
                        nc.tensor.matmul(
                            st[:, QCH : 2 * QCH],
                            lhsT=kT[p][DH:P, kb * P : (kb + 1) * P],
                            rhs=qT[p][DH:P, q0 : q0 + QCH],
                            start=True,
                            stop=True,
                        )
                        et = ebuf.tile([P, 2 * QCH], BF, tag="E")
                        nc.scalar.activation(out=et, in_=st, func=EXP)
                        e_ts.append(et)
                    if g == 0:
                        # previous pair's deferred qh1 broadcast+muls
                        flush_norm()
                    if g == MT + 4:
                        # this pair's qh0 broadcast+muls
                        flush_norm()
                    if prev is not None:
                        emit_attv(*prev)
                        if prev[0][-1][1] == MT - 1:
                            emit_norm_chain(prev[0][-1][0])
                    if wqk_next is not None:
                        if g == gq:
                            emit_qk_task(p + 1, "q", wqk_next[0])
                        if g == gk:
                            emit_qk_task(p + 1, "k", wqk_next[1])
                    prev = (us, e_ts)
                emit_attv(*prev)
                if prev[0][-1][1] == MT - 1:
                    emit_norm_chain(prev[0][-1][0])
            while pending_norm:
                flush_norm()

            # ---------------- projection: out = ao^T.T wp + bp -------
            # kt order is pair-completion order, so the scheduler can
            # run the first KT-1 accumulation steps of each output tile
            # during the last pair's attention.
            for c in range(NVCH):
                for mt in range(MT):
                    ps_p = psO.tile([P, QCH], F32, tag="O")
                    for kt in range(KT):
                        nc.tensor.matmul(
                            ps_p[:, 0:VCH],
                            lhsT=ao_sb[:, kt, mt * P : (mt + 1) * P],
                            rhs=wp_sb[:, c, kt, :],
                            start=(kt == 0),
                            stop=False,
                        )
                    nc.tensor.matmul(
                        ps_p[:, 0:VCH],
                        lhsT=ones_bf[0:1, 0:P],
                        rhs=bp_sb[0:1, c * VCH : (c + 1) * VCH],
                        start=False,
                        stop=True,
                    )
                    o_sb = outp.tile([P, VCH], F32, tag="o")
                    nc.vector.tensor_copy(
                        out=o_sb[:, 0:VCH], in_=ps_p[:, 0:VCH]
                    )
                    nc.sync.dma_start(
                        out=out_d[
                            mt * P : (mt + 1) * P,
                            c * VCH : (c + 1) * VCH,
                        ],
                        in_=o_sb[:, 0:VCH],
                    )

    return nc


# ---------------------------------------------------------------------------
# host-side layout prep
# ---------------------------------------------------------------------------

def _round_f32r(x):
    """RNE to f32r's 11-explicit-mantissa-bit grid (matches HW rounding)."""
    u = np.ascontiguousarray(x, np.float32).view(np.uint32)
    u = ((u + np.uint32(1 << 11)) >> 12) << 12
    return u.view(np.float32)


def _tile_rows(w):
    """[D, N] -> [P, (D//P) * N] with [p][kt][n] layout."""
    Dd, N = w.shape
    KT = Dd // P
    return np.ascontiguousarray(
        w.reshape(KT, P, N).transpose(1, 0, 2).reshape(P, KT * N)
    )


def host_prep_shared(w_qkv, b_qkv, w_proj, b_proj, D, H):
    """Split/retile the weights once for all cores."""
    NPAIR = H // 2
    VCH = min(512, D)
    NVCH = D // VCH

    wq3 = w_qkv.reshape(D, H, DH, 3)
    wq = np.ascontiguousarray(wq3[:, :, :, 0].reshape(D, D))
    wk = np.ascontiguousarray(wq3[:, :, :, 1].reshape(D, D))
    wv = np.ascontiguousarray(wq3[:, :, :, 2].reshape(D, D))
    wp = np.ascontiguousarray(np.asarray(w_proj, np.float32))

    NPBF = mybir.dt.np(mybir.dt.bfloat16)

    def pair_major(w):  # [D, D] -> [P, NPAIR*KT*P] fp16, pair-block major
        blocks = [
            _tile_rows(w[:, p * P : (p + 1) * P]) for p in range(NPAIR)
        ]
        return np.concatenate(blocks, axis=1).astype(np.float16)

    def chunk_major(w):  # [D, D] -> [P, NVCH*KT*VCH] bf16, chunk major
        blocks = [
            _tile_rows(w[:, c * VCH : (c + 1) * VCH]) for c in range(NVCH)
        ]
        return np.concatenate(blocks, axis=1).astype(NPBF)

    out = {
        "wq": pair_major(wq),
        "wk": pair_major(wk),
        "wv": chunk_major(wv),
        "wp": chunk_major(wp),
    }
    b3 = np.asarray(b_qkv, np.float32).reshape(H, DH, 3)
    bq = np.ascontiguousarray(b3[:, :, 0].reshape(D))
    bk = np.ascontiguousarray(b3[:, :, 1].reshape(D))
    bv = np.ascontiguousarray(b3[:, :, 2].reshape(D))
    out["bq"] = np.ascontiguousarray(bq.reshape(NPAIR, P).T).astype(np.float32)
    out["bk"] = np.ascontiguousarray(bk.reshape(NPAIR, P).T).astype(np.float32)
    out["bv"] = bv.reshape(1, D).astype(NPBF)
    out["bp"] = np.asarray(b_proj, np.float32).reshape(1, D).astype(NPBF)
    return out


def host_prep_x(x_b, TOK, D):
    """One batch element [TOK, D] -> {x: f32r, xv: bf16} tiled [P, KT*TOK]."""
    xT = np.ascontiguousarray(np.asarray(x_b, np.float32).T)  # [D, TOK]
    t = _tile_rows(xT)
    return {
        "x": t.astype(np.float16),
        "xv": t.astype(mybir.dt.np(mybir.dt.bfloat16)),
    }


# ---------------------------------------------------------------------------
# entry point
# ---------------------------------------------------------------------------

_BUILT = {}


def _get_nc(TOK, D, H, att_scale):
    key = (TOK, D, H, att_scale)
    if key not in _BUILT:
        nc = bacc.Bacc(
            "TRN2",
            target_bir_lowering=False,
            debug=False,
            dynamic_dma_scratch_size=512,
        )
        build(nc, TOK, D, H, att_scale)
        nc.compile()
        nc.finalize()
        _BUILT[key] = nc
    return _BUILT[key]


def kernel(x, w_qkv, b_qkv, w_proj, b_proj):
    from concourse.bass_utils import run_bass_kernel_spmd

    x = np.asarray(x, np.float32)
    B, TOK, D = x.shape
    H = H_FULL
    shared = host_prep_shared(
        np.asarray(w_qkv, np.float32),
        np.asarray(b_qkv, np.float32),
        np.asarray(w_proj, np.float32),
        np.asarray(b_proj, np.float32),
        D,
        H,
    )
    in_maps = []
    for b in range(B):
        m = dict(shared)
        m.update(host_prep_x(x[b], TOK, D))
        in_maps.append(m)

    nc = _get_nc(TOK, D, H, ATT_SCALE_FULL)
    res = run_bass_kernel_spmd(nc, in_maps, list(range(N_CORES)))
    out = np.stack([res.results[b]["out"] for b in range(B)], axis=0)
    return out.astype(np.float32)
